# revision 28
# baseline (speedup 1.0000x reference)
"""Trainium2 Bass kernel for nn_AttentionBlock (B=8, C=512, H=W=32, NH=8, DH=64).

Sharding: pure data-parallel — one batch element per NeuronCore (8 cores).
Per-core pipeline (channels-on-partitions layout, HW=1024 spatial):
  groupnorm -> qkv 1x1conv (fp8 DoubleRow matmul) -> attention:
    scores computed transposed (pT[j,i] = exp(k_j . q_i / 8), exp split
    between ScalarE ACT and a custom DVE EXP64 op, no softmax reductions),
    then out2T[d,i] = V^T-stationary matmul streaming pT, row sums via a
    ones-column in V, transpose via DMA xbar, normalize on GpSimd
    -> reshape via DRAM round-trip -> proj 1x1conv (bf16) -> residual.
Software-pipelined over head pairs; conv tiles fill pair 0's attnv slot,
proj partials fill the drain slot.

v2 changes vs v1 (148.9us):
  - x DMA split per half-tile + groupnorm starts per-tile on arrival
  - qkv convs in fp8e4 DoubleRow (half the matmul count)
  - qk bias add + attn-out PSUM->SBUF cast moved to ScalarE (ACT identity/copy)
  - o2 normalize + xpb residual-bias adds moved to GpSimd
  - ACT table loads forced to (sqrt, exp) order once each
  - scores matmuls head-major for LDWEIGHTS adjacency
"""

import numpy as np
import ml_dtypes

import concourse.bass as bass
import concourse.mybir as mybir
import concourse.tile as tile
from concourse import bacc
from concourse.bass_utils import run_bass_kernel_spmd

F32 = mybir.dt.float32
BF16 = mybir.dt.bfloat16
FP8E4 = mybir.dt.float8e4
FP8E5 = mybir.dt.float8e5

B, C, HW = 8, 512, 1024
NH, DH = 8, 64
GROUPS, EPS = 32, 1e-5
CT = C // 128          # 4 channel tiles
ST = HW // 128         # 8 spatial tiles
GPT = 8                # groups per 128-channel tile
CPG = 16               # channels per group

FP8_CONV = True        # qkv conv in fp8e4 DoubleRow (proj stays bf16)

_CACHE: dict = {}


def _register_exp64():
    """Register the custom DVE op  out = (1 + in0*s0)^64  (approx exp(in0*s0*64)).

    1 mult + 1 add + 6 squarings = 8 ALU stages (the DVE datapath limit).
    The uop table is generated per-NEFF at compile time, no firmware change.
    """
    from concourse import dve_ops as DO
    if "EXP64_ANT" in DO._SUB_OPCODE_FOR_NAME:
        return next(op for op in DO.OPS if op.name == "EXP64_ANT")
    from concourse.dve_spec import Spec, Src0, C0, One, sq, lower, _has_src1
    from concourse.dve_uop import DveOpSpec

    body = sq(sq(sq(sq(sq(sq(One + Src0 * C0))))))
    spec = Spec(
        body=body,
        reference=lambda in0, in1, s0, s1, imm2:
            (1.0 + in0.astype(np.float32) * np.float32(s0)) ** 64,
    )
    row = max(DO._SUB_OPCODE_FOR_NAME.values()) + 1
    shas = {}
    for ver in ("v3", "v4"):
        try:
            u = lower(spec, ver=ver)
            shas[ver] = DveOpSpec(
                name="EXP64_ANT", opcode=row, uops=u, rd1_en=_has_src1(spec)
            ).sha(ver)
        except Exception:
            pass
    op = DO.DveOp("EXP64_ANT", spec, subdim=False, uops_sha=shas)
    DO.OPS.append(op)
    DO._SUB_OPCODE_FOR_NAME["EXP64_ANT"] = row
    DO.CUSTOM_DVE_SPECS["EXP64_ANT"] = spec
    return op


def _build():
    EXP64 = _register_exp64()
    nc = bacc.Bacc("TRN2", target_bir_lowering=False, debug=False, num_devices=8)

    WQDT = FP8E4 if FP8_CONV else BF16
    x_d = nc.declare_dram_parameter("x", [C, HW], BF16, isOutput=False)
    wq_d = nc.declare_dram_parameter("wqkvT", [C, 3 * C], WQDT, isOutput=False)
    wp_d = nc.declare_dram_parameter("wprojT", [C, C], BF16, isOutput=False)
    # all small per-partition constants packed into one [128, 28] array:
    # cols 0-3 gamma, 4-7 beta, 8-15 qkb, 16-19 proj_b, 20-27 G
    cpk_d = nc.declare_dram_parameter("cpack", [128, 28], F32, isOutput=False)
    vb_d = nc.declare_dram_parameter("vb", [C], BF16, isOutput=False)
    GT_d = nc.declare_dram_parameter("GT", [8, 128], F32, isOutput=False)
    out_d = nc.declare_dram_parameter("out", [C, HW], BF16, isOutput=True)
    h2_d = nc.dram_tensor("h2d", [C, HW], BF16)

    import bass_rust
    from contextlib import ExitStack

    with tile.TileContext(nc) as tc, ExitStack() as ctx:
        const = ctx.enter_context(tc.tile_pool(name="const", bufs=1))
        small = ctx.enter_context(tc.tile_pool(name="small", bufs=2))
        xp = ctx.enter_context(tc.tile_pool(name="xp", bufs=1))
        hp = ctx.enter_context(tc.tile_pool(name="hp", bufs=1))
        wqp = ctx.enter_context(tc.tile_pool(name="wqp", bufs=1))
        wpp = ctx.enter_context(tc.tile_pool(name="wpp", bufs=1))
        qkp = ctx.enter_context(tc.tile_pool(name="qkp", bufs=1))
        vpl = ctx.enter_context(tc.tile_pool(name="vpl", bufs=1))
        ptp = ctx.enter_context(tc.tile_pool(name="ptp", bufs=4))
        o2tp = ctx.enter_context(tc.tile_pool(name="o2tp", bufs=2))
        o2trp = ctx.enter_context(tc.tile_pool(name="o2trp", bufs=2))
        o2p = ctx.enter_context(tc.tile_pool(name="o2p", bufs=2))
        h2p = ctx.enter_context(tc.tile_pool(name="h2p", bufs=1))
        outp = ctx.enter_context(tc.tile_pool(name="outp", bufs=4))
        # PSUM: psA = scores (+ proj at drain), psB = convs/attnv; 8 banks
        psA = ctx.enter_context(tc.tile_pool(name="psA", bufs=2, space="PSUM"))
        psB = ctx.enter_context(tc.tile_pool(name="psB", bufs=2, space="PSUM"))

        dummy = small.tile([1, 1], F32, tag="dummy")
        nc.gpsimd.memset(dummy[:], 1.0)

        # ACT table preloads: sqrt now, exp forced after the last sqrt
        dummy2 = small.tile([1, 1], F32, tag="dummy2")
        nc.scalar.activation(dummy2[:], dummy[:],
                             mybir.ActivationFunctionType.Sqrt, bias=0.0, scale=1.0)

        cpk_sb = const.tile([128, 28], F32, tag="cpk")
        nc.scalar.dma_start(out=cpk_sb[:], in_=cpk_d[:])
        gam_col = lambda t: cpk_sb[:, 0 + t:1 + t]
        bet_col = lambda t: cpk_sb[:, 4 + t:5 + t]
        qkb_col = lambda m: cpk_sb[:, 8 + m:9 + m]
        pb_col = lambda o: cpk_sb[:, 16 + o:17 + o]
        GT_sb = const.tile([8, 128], F32, tag="GT")
        nc.scalar.dma_start(out=GT_sb[:], in_=GT_d[:])
        vb_sb = const.tile([1, C], BF16, tag="vb")
        nc.scalar.dma_start(out=vb_sb[:], in_=vb_d[:].rearrange("c -> () c"))
        ones1 = const.tile([1, 128], BF16, tag="ones1")
        nc.vector.memset(ones1[:], 1.0)
        zeros1 = const.tile([1, 128], BF16, tag="zeros1")
        nc.vector.memset(zeros1[:], 0.0)

        # ---- bulk input DMAs: full x tiles (4KB lines) spread over all
        # three DMA-capable engines; wq after; wp deferred to mid-kernel ----
        x_sb = xp.tile([128, CT, HW], BF16)
        x_r = x_d[:].rearrange("(t p) s -> t p s", p=128)
        x_eng = [nc.sync, nc.scalar, nc.gpsimd, nc.sync]
        for t in range(CT):
            x_eng[t].dma_start(out=x_sb[:, t, :], in_=x_r[t])

        wq_sb = wqp.tile([128, CT, 3 * C], WQDT)
        wq_r = wq_d[:].rearrange("(t p) o -> t p o", p=128)
        for k in range(CT):
            nc.gpsimd.dma_start(out=wq_sb[:, k, :], in_=wq_r[k])
        wp_sb = wpp.tile([128, CT, C], BF16)

        # ---- per-tile groupnorm (starts as each x tile arrives) ----
        eps_sb = small.tile([8, 1], F32, tag="eps")
        nc.vector.memset(eps_sb[:], float(EPS))
        HDT = FP8E4 if FP8_CONV else BF16
        h_sb = hp.tile([128, CT, HW], HDT)
        mv = small.tile([128, CT, 3], F32, tag="mv")
        last_sqrt = None
        for t in range(CT):
            st = small.tile([128, 2, 6], F32, tag="bnst")
            x3 = x_sb[:, t, :].rearrange("p (a f) -> p a f", a=2)
            nc.vector.bn_stats(st[:, 0, :], x3[:, 0, :])
            nc.vector.bn_stats(st[:, 1, :], x3[:, 1, :])
            nc.vector.bn_aggr(mv[:, t, 0:2], st[:])
            nc.vector.tensor_mul(mv[:, t, 2:3], mv[:, t, 0:1], mv[:, t, 0:1])
            psg = psB.tile([8, 3], F32, tag="att", name=f"g_{t}")
            nc.tensor.matmul(psg[:], lhsT=cpk_sb[:, 20:28], rhs=mv[:, t, :],
                             start=True, stop=True)
            gst = small.tile([8, 3], F32, tag="gst")
            nc.vector.tensor_copy(gst[:], psg[:])
            sqv = small.tile([8, 2], F32, tag="sqv")
            nc.vector.tensor_mul(sqv[:, 0:1], gst[:, 0:1], gst[:, 0:1])
            nc.vector.tensor_add(sqv[:, 1:2], gst[:, 1:2], gst[:, 2:3])
            nc.vector.tensor_sub(sqv[:, 1:2], sqv[:, 1:2], sqv[:, 0:1])
            srt = small.tile([8, 1], F32, tag="srt")
            last_sqrt = nc.scalar.activation(
                srt[:], sqv[:, 1:2], mybir.ActivationFunctionType.Sqrt,
                bias=eps_sb[:], scale=1.0)
            rstd = small.tile([8, 1], F32, tag="rstd")
            nc.vector.reciprocal(rstd[:], srt[:])
            gv2 = small.tile([8, 2], F32, tag="gv2")
            nc.vector.tensor_copy(gv2[:, 0:1], rstd[:])
            nc.vector.tensor_copy(gv2[:, 1:2], gst[:, 0:1])
            bc_ps = psB.tile([128, 2], F32, tag="att", name=f"bc_{t}")
            nc.tensor.matmul(bc_ps[:], lhsT=GT_sb[:], rhs=gv2[:],
                             start=True, stop=True)
            sc = small.tile([128, CT, 2], F32, tag="sc")
            nc.vector.tensor_mul(sc[:, t, 0:1], bc_ps[:, 0:1], gam_col(t))
            nc.vector.tensor_mul(sc[:, t, 1:2], bc_ps[:, 1:2], sc[:, t, 0:1])
            nc.vector.tensor_sub(sc[:, t, 1:2], bet_col(t), sc[:, t, 1:2])
            # apply per half, Vector + GpSimd in parallel
            for n, eng in ((0, nc.vector), (1, nc.gpsimd)):
                eng.tensor_scalar(
                    out=h_sb[:, t, n * 512:(n + 1) * 512],
                    in0=x_sb[:, t, n * 512:(n + 1) * 512],
                    scalar1=sc[:, t, 0:1], scalar2=sc[:, t, 1:2],
                    op0=mybir.AluOpType.mult, op1=mybir.AluOpType.add)
        # preload ACT exp table after the last sqrt (forced order so Tile
        # can't hoist it between the sqrts and thrash the table RAM)
        dummy3 = small.tile([1, 1], F32, tag="dummy3")
        expd = nc.scalar.activation(dummy3[:], dummy[:],
                                    mybir.ActivationFunctionType.Exp, scale=1.0)
        bass_rust.add_dep_helper(expd.ins, last_sqrt.ins, reason="ACT table order")

        qk_sb = qkp.tile([128, 2 * CT, HW], BF16)
        v_sb = vpl.tile([128, ST, NH * 66], FP8E4)
        nc.vector.memset(
            v_sb[:].rearrange("p m (h e) -> p m h e", e=66)[:, :, :, 64], 1.0)
        h2_sb = h2p.tile([128, CT, HW], BF16)

        if FP8_CONV:
            def emit_qk_conv(m):
                ps = psB.tile([128, HW], F32, tag="att", name=f"qkps{m}")
                for kk in (0, 2):
                    for n in range(2):
                        nc.tensor.matmul(
                            ps[:, n * 512:(n + 1) * 512],
                            lhsT=wq_sb[:, kk:kk + 2, m * 128:(m + 1) * 128],
                            rhs=h_sb[:, kk:kk + 2, n * 512:(n + 1) * 512],
                            start=(kk == 0), stop=(kk == 2),
                            perf_mode=mybir.MatmulPerfMode.DoubleRow)
                nc.scalar.add(qk_sb[:, m, :], ps[:], add=qkb_col(m))

            def emit_v_conv(m):
                psv = psB.tile([128, 512], F32, tag="att", name=f"vps{m}")
                for kk in (0, 2):
                    nc.tensor.matmul(
                        psv[:],
                        lhsT=h_sb[:, kk:kk + 2, m * 128:(m + 1) * 128],
                        rhs=wq_sb[:, kk:kk + 2, 2 * C:3 * C],
                        start=(kk == 0), stop=False,
                        perf_mode=mybir.MatmulPerfMode.DoubleRow)
                # rank-1 v-bias add: psv += ones[s] * vb[vc]
                nc.tensor.matmul(psv[:], lhsT=ones1[:], rhs=vb_sb[:],
                                 start=False, stop=True)
                nc.vector.tensor_copy(
                    v_sb[:, m, :].rearrange("p (h e) -> p h e", e=66)[:, :, 0:64],
                    psv[:].rearrange("p (h d) -> p h d", d=64))
        else:
            def emit_qk_conv(m):
                ps = psB.tile([128, HW], F32, tag="att", name=f"qkps{m}")
                for k in range(CT):
                    for n in range(2):
                        nc.tensor.matmul(
                            ps[:, n * 512:(n + 1) * 512],
                            lhsT=wq_sb[:, k, m * 128:(m + 1) * 128],
                            rhs=h_sb[:, k, n * 512:(n + 1) * 512],
                            start=(k == 0), stop=(k == CT - 1))
                nc.scalar.add(qk_sb[:, m, :], ps[:], add=qkb_col(m))

            def emit_v_conv(m):
                psv = psB.tile([128, 512], F32, tag="att", name=f"vps{m}")
                for k in range(CT):
                    nc.tensor.matmul(
                        psv[:],
                        lhsT=h_sb[:, k, m * 128:(m + 1) * 128],
                        rhs=wq_sb[:, k, 2 * C:3 * C],
                        start=(k == 0), stop=False)
                nc.tensor.matmul(psv[:], lhsT=ones1[:], rhs=vb_sb[:],
                                 start=False, stop=True)
                nc.vector.tensor_copy(
                    v_sb[:, m, :].rearrange("p (h e) -> p h e", e=66)[:, :, 0:64],
                    psv[:].rearrange("p (h d) -> p h d", d=64))

        # q/k tiles for pair 0 first, so its scores can start immediately
        emit_qk_conv(0)
        emit_qk_conv(4)
        # remaining conv work, interleaved into pair 0's attnv slot below
        conv_work = [lambda m=m: emit_qk_conv(m) for m in (1, 5, 2, 6, 3, 7)]
        conv_work += [lambda m=m: emit_v_conv(m) for m in range(ST)]

        def emit_scores_step(cur, step):
            pss = []
            for (h, pt) in cur:
                base = 64 * (h % 2)
                ps = psA.tile([128, HW], F32, tag="sc", name=f"scps{h}_{step}")
                pss.append(ps)
                kT = qk_sb[base:base + 64, CT + h // 2,
                           step * 128:(step + 1) * 128]
                qT = qk_sb[base:base + 64, h // 2, :]
                for n in range(2):
                    nc.tensor.matmul(
                        ps[:, n * 512:(n + 1) * 512], lhsT=kT,
                        rhs=qT[:, n * 512:(n + 1) * 512],
                        start=True, stop=True)
            (hA, ptA), (hB, ptB) = cur
            expi = nc.scalar.activation(
                ptA[:, step, :], pss[0][:],
                mybir.ActivationFunctionType.Exp,
                scale=float(DH ** -0.5))
            if hA == 0 and step == 0:
                first_exp.append(expi)
            nc.vector._custom_dve(
                EXP64, out=ptB[:, step, :], in0=pss[1][:],
                s0=float(DH ** -0.5) / 64.0)

        def emit_attnv_sm(h, pt, sm, state, last=False):
            # DoubleRow packs a j-tile pair per matmul (fp8 weights
            # 2-per-cell, K=256 virtual)
            if sm == 0:
                state[h] = psB.tile([128, HW], F32, tag="att", name=f"po{h}")
            po = state[h]
            jj = 2 * sm
            v2_ = v_sb[:].rearrange(
                "p m (hh e) -> p m hh e", e=66)[:, jj:jj + 2, h, 0:65]
            for n in range(2):
                nc.tensor.matmul(
                    po[0:65, n * 512:(n + 1) * 512],
                    lhsT=v2_,
                    rhs=pt[:, jj:jj + 2, n * 512:(n + 1) * 512],
                    start=(sm == 0), stop=(sm == 3),
                    perf_mode=mybir.MatmulPerfMode.DoubleRow)
            if sm == 3:
                o2t = o2tp.tile([80, HW], BF16, tag="o2t")
                if h % 2 == 0:
                    nc.scalar.copy(o2t[0:65, :], po[0:65, :])
                else:
                    nc.vector.tensor_copy(o2t[0:65, :], po[0:65, :])
                o2tr = o2trp.tile([128, ST, 80], BF16, tag="o2tr")
                nc.sync.dma_start_transpose(o2tr[:], o2t[:])
                linv = small.tile([128, ST], F32, tag="linv")
                nc.vector.reciprocal(linv[:], o2tr[:, :, 64])
                o2 = o2p.tile([128, 512], BF16, tag="o2")
                lap = linv[:]
                lbc = bass.AP(tensor=lap.tensor, offset=lap.offset,
                              ap=[[lap.ap[0][0], 128], [1, ST], [0, 64]])
                nc.gpsimd.tensor_mul(
                    o2[:].rearrange("p (q d) -> p q d", d=64),
                    o2tr[:, :, 0:64], lbc)
                # alternate h2 round-trip DMA queues by head parity so the
                # two heads of a pair don't serialize on one DGE queue; the
                # round-trip goes in halves so the readback pipelines with
                # the write
                dmae = nc.sync if (h % 2 == 0) != last else nc.gpsimd
                k, half = h // 2, h % 2
                h2f = h2_d[:].rearrange("c s -> (c s)")
                o2q = o2[:].rearrange("p (q d) -> p q d", d=64)
                for qh in range(2):
                    wr = dmae.dma_start(
                        out=h2f[h * 65536 + qh * 32768:
                                h * 65536 + (qh + 1) * 32768]
                        .rearrange("(q p d) -> p q d", p=128, d=64),
                        in_=o2q[:, qh * 4:(qh + 1) * 4, :])
                    rd = dmae.dma_start(
                        out=h2_sb[64 * half + 32 * qh:64 * half + 32 * qh + 32,
                                  k, :],
                        in_=h2_d[h * 64 + 32 * qh:h * 64 + 32 * qh + 32, :])
                    bass_rust.add_dep_helper(rd.ins, wr.ins, reason="h2 RAW")

        proj_pp = {}

        def emit_proj(o, ks, finish, pool=None, tag="sc"):
            if o not in proj_pp:
                proj_pp[o] = (pool or psA).tile([128, HW], F32, tag=tag,
                                                name=f"pp{o}")
            pp = proj_pp[o]
            for k in ks:
                for n in range(2):
                    nc.tensor.matmul(
                        pp[:, n * 512:(n + 1) * 512],
                        lhsT=wp_sb[:, k, o * 128:(o + 1) * 128],
                        rhs=h2_sb[:, k, n * 512:(n + 1) * 512],
                        start=(k == 0), stop=(k == CT - 1))
            if finish:
                # out = (proj + proj_b) + x  in one fused DVE op
                ot = outp.tile([128, HW], BF16, tag="ot")
                nc.vector.scalar_tensor_tensor(
                    out=ot[:], in0=pp[:], scalar=pb_col(o),
                    in1=x_sb[:, o, :],
                    op0=mybir.AluOpType.add, op1=mybir.AluOpType.add)
                eng = nc.sync if o % 2 == 0 else nc.scalar
                eng.dma_start(out=out_d[o * 128:(o + 1) * 128, :], in_=ot[:])
                del proj_pp[o]

        # ---- attention pair loop (software pipelined, pairs 0-3) ----
        first_exp = []
        prev = None
        for hp_i in range(4):
            hA, hB = 2 * hp_i, 2 * hp_i + 1
            ptA = ptp.tile([128, ST, HW], FP8E5, tag="pt", name=f"pt{hA}")
            ptB = ptp.tile([128, ST, HW], FP8E5, tag="pt", name=f"pt{hB}")
            cur = [(hA, ptA), (hB, ptB)]
            state = {}
            for step in range(8):
                emit_scores_step(cur, step)
                if prev is not None:
                    h, pt = prev[step // 4]
                    emit_attnv_sm(h, pt, step % 4, state)
                elif conv_work:
                    # pair 0: fill the attnv slot with remaining conv tiles
                    conv_work.pop(0)()
                    if conv_work and step % 2 == 1:
                        conv_work.pop(0)()
            while prev is None and conv_work:
                conv_work.pop(0)()
            if hp_i == 0:
                # proj weights DMA deferred past the input-load window (wp
                # isn't needed until the drain); dep stops Tile hoisting it
                wpdma = nc.gpsimd.dma_start(
                    out=wp_sb[:],
                    in_=wp_d[:].rearrange("(t p) o -> p t o", p=128))
                bass_rust.add_dep_helper(wpdma.ins, first_exp[0].ins,
                                         reason="defer wp load")
            prev = cur

        # ---- drain: last pair's attnv at double rate, proj spread under
        # the h2 round-trip latency so the PE stays warm ----
        (h6, pt6), (h7, pt7) = prev
        state = {}
        for sm in range(4):
            emit_attnv_sm(h6, pt6, sm, state, last=True)
            emit_attnv_sm(h7, pt7, sm, state, last=True)
            if sm == 1:
                emit_proj(0, [0], finish=False)
            if sm == 2:
                emit_proj(1, [0], finish=False)
        emit_proj(0, [1], finish=False)
        emit_proj(1, [1], finish=False)
        emit_proj(0, [2], finish=False)
        emit_proj(1, [2], finish=False)
        emit_proj(2, [0], finish=False, pool=psB, tag="att")
        emit_proj(2, [1], finish=False, pool=psB, tag="att")
        # the rest is emitted after the sm3 normalize chains above, so these
        # matmuls execute during the h2 DRAM round-trip and keep the PE warm
        emit_proj(3, [0], finish=False, pool=psB, tag="att")
        emit_proj(2, [2], finish=False, pool=psB, tag="att")
        emit_proj(3, [1], finish=False, pool=psB, tag="att")
        emit_proj(3, [2], finish=False, pool=psB, tag="att")

        # ---- proj finish ----
        emit_proj(0, [3], finish=True)
        emit_proj(1, [3], finish=True)
        emit_proj(2, [3], finish=True)
        emit_proj(3, [3], finish=True)

    nc.compile()
    return nc


def _host_prep(x, norm_gamma, norm_beta, qkv_w, qkv_b, proj_w, proj_b):
    x = np.asarray(x, dtype=np.float32).reshape(B, C, HW)
    qkv_w = np.asarray(qkv_w, dtype=np.float32)
    qkv_b = np.asarray(qkv_b, dtype=np.float32)
    proj_w = np.asarray(proj_w, dtype=np.float32)
    proj_b = np.asarray(proj_b, dtype=np.float32)

    wq_np = np.ascontiguousarray(qkv_w.T)
    if FP8_CONV:
        wqkvT = np.clip(wq_np, -440.0, 440.0).astype(ml_dtypes.float8_e4m3fn)
    else:
        wqkvT = wq_np.astype(ml_dtypes.bfloat16)
    wprojT = np.ascontiguousarray(proj_w.T).astype(ml_dtypes.bfloat16)
    qkb = np.ascontiguousarray(qkv_b[:2 * C])
    vb = np.ascontiguousarray(qkv_b[2 * C:]).astype(ml_dtypes.bfloat16)
    pb = np.ascontiguousarray(proj_b, dtype=np.float32)

    G = np.zeros((128, GPT), np.float32)
    for p in range(128):
        G[p, p // CPG] = 1.0 / CPG
    GT = np.zeros((8, 128), np.float32)
    for p in range(128):
        GT[p // CPG, p] = 1.0

    gamma = np.asarray(norm_gamma, dtype=np.float32)
    beta = np.asarray(norm_beta, dtype=np.float32)
    # packed per-partition constants [128, 28]:
    # cols 0-3 gamma, 4-7 beta, 8-15 qkb, 16-19 proj_b, 20-27 G
    cpack = np.zeros((128, 28), np.float32)
    cpack[:, 0:4] = gamma.reshape(CT, 128).T
    cpack[:, 4:8] = beta.reshape(CT, 128).T
    cpack[:, 8:16] = qkb.reshape(2 * CT, 128).T
    cpack[:, 16:20] = pb.reshape(CT, 128).T
    cpack[:, 20:28] = G
    cpack = np.ascontiguousarray(cpack)

    in_maps = []
    for b in range(B):
        in_maps.append({
            "x": np.ascontiguousarray(x[b]).astype(ml_dtypes.bfloat16),
            "wqkvT": wqkvT, "wprojT": wprojT,
            "cpack": cpack, "vb": vb, "GT": GT,
        })
    return in_maps


def _run(inputs: dict, trace: bool = False, tmpdir=None):
    if "nc" not in _CACHE:
        _CACHE["nc"] = _build()
    nc = _CACHE["nc"]
    in_maps = _host_prep(**inputs)
    res = run_bass_kernel_spmd(nc, in_maps, core_ids=list(range(8)), trace=trace,
                               tmpdir=tmpdir)
    out = np.stack([np.asarray(r["out"]).astype(np.float32)
                    for r in res.results]).reshape(B, C, 32, 32)
    return out, res


def kernel(**inputs):
    out, _ = _run(inputs, trace=False)
    return out


# revision 29
# speedup vs baseline: 1.0161x; 1.0161x over previous
"""Trainium2 Bass kernel for nn_AttentionBlock (B=8, C=512, H=W=32, NH=8, DH=64).

Sharding: pure data-parallel — one batch element per NeuronCore (8 cores).
Per-core pipeline (channels-on-partitions layout, HW=1024 spatial):
  groupnorm -> qkv 1x1conv (fp8 DoubleRow matmul) -> attention:
    scores computed transposed (pT[j,i] = exp(k_j . q_i / 8), exp split
    between ScalarE ACT and a custom DVE EXP64 op, no softmax reductions),
    then out2T[d,i] = V^T-stationary matmul streaming pT, row sums via a
    ones-column in V, transpose via DMA xbar, normalize on GpSimd
    -> reshape via DRAM round-trip -> proj 1x1conv (bf16) -> residual.
Software-pipelined over head pairs; conv tiles fill pair 0's attnv slot,
proj partials fill the drain slot.

v2 changes vs v1 (148.9us):
  - x DMA split per half-tile + groupnorm starts per-tile on arrival
  - qkv convs in fp8e4 DoubleRow (half the matmul count)
  - qk bias add + attn-out PSUM->SBUF cast moved to ScalarE (ACT identity/copy)
  - o2 normalize + xpb residual-bias adds moved to GpSimd
  - ACT table loads forced to (sqrt, exp) order once each
  - scores matmuls head-major for LDWEIGHTS adjacency
"""

import numpy as np
import ml_dtypes

import concourse.bass as bass
import concourse.mybir as mybir
import concourse.tile as tile
from concourse import bacc
from concourse.bass_utils import run_bass_kernel_spmd

F32 = mybir.dt.float32
BF16 = mybir.dt.bfloat16
FP8E4 = mybir.dt.float8e4
FP8E5 = mybir.dt.float8e5

B, C, HW = 8, 512, 1024
NH, DH = 8, 64
GROUPS, EPS = 32, 1e-5
CT = C // 128          # 4 channel tiles
ST = HW // 128         # 8 spatial tiles
GPT = 8                # groups per 128-channel tile
CPG = 16               # channels per group

FP8_CONV = True        # qkv conv in fp8e4 DoubleRow (proj stays bf16)

_CACHE: dict = {}


def _register_exp64():
    """Register the custom DVE op  out = (1 + in0*s0)^64  (approx exp(in0*s0*64)).

    1 mult + 1 add + 6 squarings = 8 ALU stages (the DVE datapath limit).
    The uop table is generated per-NEFF at compile time, no firmware change.
    """
    from concourse import dve_ops as DO
    if "EXP64_ANT" in DO._SUB_OPCODE_FOR_NAME:
        return next(op for op in DO.OPS if op.name == "EXP64_ANT")
    from concourse.dve_spec import Spec, Src0, C0, One, sq, lower, _has_src1
    from concourse.dve_uop import DveOpSpec

    body = sq(sq(sq(sq(sq(sq(One + Src0 * C0))))))
    spec = Spec(
        body=body,
        reference=lambda in0, in1, s0, s1, imm2:
            (1.0 + in0.astype(np.float32) * np.float32(s0)) ** 64,
    )
    row = max(DO._SUB_OPCODE_FOR_NAME.values()) + 1
    shas = {}
    for ver in ("v3", "v4"):
        try:
            u = lower(spec, ver=ver)
            shas[ver] = DveOpSpec(
                name="EXP64_ANT", opcode=row, uops=u, rd1_en=_has_src1(spec)
            ).sha(ver)
        except Exception:
            pass
    op = DO.DveOp("EXP64_ANT", spec, subdim=False, uops_sha=shas)
    DO.OPS.append(op)
    DO._SUB_OPCODE_FOR_NAME["EXP64_ANT"] = row
    DO.CUSTOM_DVE_SPECS["EXP64_ANT"] = spec
    return op


def _build():
    EXP64 = _register_exp64()
    nc = bacc.Bacc("TRN2", target_bir_lowering=False, debug=False, num_devices=8)

    WQDT = FP8E4 if FP8_CONV else BF16
    x_d = nc.declare_dram_parameter("x", [C, HW], BF16, isOutput=False)
    wq_d = nc.declare_dram_parameter("wqkvT", [C, 3 * C], WQDT, isOutput=False)
    wp_d = nc.declare_dram_parameter("wprojT", [C, C], BF16, isOutput=False)
    # all small per-partition constants packed into one [128, 28] array:
    # cols 0-3 gamma, 4-7 beta, 8-15 qkb, 16-19 proj_b, 20-27 G
    cpk_d = nc.declare_dram_parameter("cpack", [128, 28], F32, isOutput=False)
    vb_d = nc.declare_dram_parameter("vb", [C], BF16, isOutput=False)
    GT_d = nc.declare_dram_parameter("GT", [8, 128], F32, isOutput=False)
    out_d = nc.declare_dram_parameter("out", [C, HW], BF16, isOutput=True)
    h2_d = nc.dram_tensor("h2d", [C, HW], BF16)

    import bass_rust
    from contextlib import ExitStack

    with tile.TileContext(nc) as tc, ExitStack() as ctx:
        const = ctx.enter_context(tc.tile_pool(name="const", bufs=1))
        small = ctx.enter_context(tc.tile_pool(name="small", bufs=2))
        xp = ctx.enter_context(tc.tile_pool(name="xp", bufs=1))
        hp = ctx.enter_context(tc.tile_pool(name="hp", bufs=1))
        wqp = ctx.enter_context(tc.tile_pool(name="wqp", bufs=1))
        wpp = ctx.enter_context(tc.tile_pool(name="wpp", bufs=1))
        qkp = ctx.enter_context(tc.tile_pool(name="qkp", bufs=1))
        vpl = ctx.enter_context(tc.tile_pool(name="vpl", bufs=1))
        ptp = ctx.enter_context(tc.tile_pool(name="ptp", bufs=4))
        o2tp = ctx.enter_context(tc.tile_pool(name="o2tp", bufs=2))
        o2trp = ctx.enter_context(tc.tile_pool(name="o2trp", bufs=2))
        o2p = ctx.enter_context(tc.tile_pool(name="o2p", bufs=2))
        h2p = ctx.enter_context(tc.tile_pool(name="h2p", bufs=1))
        outp = ctx.enter_context(tc.tile_pool(name="outp", bufs=4))
        # PSUM: psA = scores (+ proj at drain), psB = convs/attnv; 8 banks
        psA = ctx.enter_context(tc.tile_pool(name="psA", bufs=2, space="PSUM"))
        psB = ctx.enter_context(tc.tile_pool(name="psB", bufs=2, space="PSUM"))

        dummy = small.tile([1, 1], F32, tag="dummy")
        nc.gpsimd.memset(dummy[:], 1.0)

        # ACT table preloads: sqrt now, exp forced after the last sqrt
        dummy2 = small.tile([1, 1], F32, tag="dummy2")
        nc.scalar.activation(dummy2[:], dummy[:],
                             mybir.ActivationFunctionType.Sqrt, bias=0.0, scale=1.0)

        cpk_sb = const.tile([128, 28], F32, tag="cpk")
        nc.scalar.dma_start(out=cpk_sb[:], in_=cpk_d[:])
        gam_col = lambda t: cpk_sb[:, 0 + t:1 + t]
        bet_col = lambda t: cpk_sb[:, 4 + t:5 + t]
        qkb_col = lambda m: cpk_sb[:, 8 + m:9 + m]
        pb_col = lambda o: cpk_sb[:, 16 + o:17 + o]
        GT_sb = const.tile([8, 128], F32, tag="GT")
        nc.scalar.dma_start(out=GT_sb[:], in_=GT_d[:])
        vb_sb = const.tile([1, C], BF16, tag="vb")
        nc.scalar.dma_start(out=vb_sb[:], in_=vb_d[:].rearrange("c -> () c"))
        ones1 = const.tile([1, 128], BF16, tag="ones1")
        nc.vector.memset(ones1[:], 1.0)
        zeros1 = const.tile([1, 128], BF16, tag="zeros1")
        nc.vector.memset(zeros1[:], 0.0)

        # ---- bulk input DMAs: full x tiles (4KB lines) spread over all
        # three DMA-capable engines; wq after; wp deferred to mid-kernel ----
        x_sb = xp.tile([128, CT, HW], BF16)
        x_r = x_d[:].rearrange("(t p) s -> t p s", p=128)
        x_eng = [nc.sync, nc.scalar, nc.gpsimd, nc.sync]
        for t in range(CT):
            x_eng[t].dma_start(out=x_sb[:, t, :], in_=x_r[t])

        wq_sb = wqp.tile([128, CT, 3 * C], WQDT)
        wq_r = wq_d[:].rearrange("(t p) o -> t p o", p=128)
        for k in range(CT):
            nc.gpsimd.dma_start(out=wq_sb[:, k, :], in_=wq_r[k])
        wp_sb = wpp.tile([128, CT, C], BF16)

        # ---- per-tile groupnorm (starts as each x tile arrives) ----
        eps_sb = small.tile([8, 1], F32, tag="eps")
        nc.vector.memset(eps_sb[:], float(EPS))
        HDT = FP8E4 if FP8_CONV else BF16
        h_sb = hp.tile([128, CT, HW], HDT)
        mv = small.tile([128, CT, 3], F32, tag="mv")
        last_sqrt = None
        for t in range(CT):
            st = small.tile([128, 2, 6], F32, tag="bnst")
            x3 = x_sb[:, t, :].rearrange("p (a f) -> p a f", a=2)
            nc.vector.bn_stats(st[:, 0, :], x3[:, 0, :])
            nc.vector.bn_stats(st[:, 1, :], x3[:, 1, :])
            nc.vector.bn_aggr(mv[:, t, 0:2], st[:])
            nc.vector.tensor_mul(mv[:, t, 2:3], mv[:, t, 0:1], mv[:, t, 0:1])
            psg = psB.tile([8, 3], F32, tag="att", name=f"g_{t}")
            nc.tensor.matmul(psg[:], lhsT=cpk_sb[:, 20:28], rhs=mv[:, t, :],
                             start=True, stop=True)
            gst = small.tile([8, 3], F32, tag="gst")
            nc.vector.tensor_copy(gst[:], psg[:])
            sqv = small.tile([8, 2], F32, tag="sqv")
            nc.vector.tensor_mul(sqv[:, 0:1], gst[:, 0:1], gst[:, 0:1])
            nc.vector.tensor_add(sqv[:, 1:2], gst[:, 1:2], gst[:, 2:3])
            nc.vector.tensor_sub(sqv[:, 1:2], sqv[:, 1:2], sqv[:, 0:1])
            srt = small.tile([8, 1], F32, tag="srt")
            last_sqrt = nc.scalar.activation(
                srt[:], sqv[:, 1:2], mybir.ActivationFunctionType.Sqrt,
                bias=eps_sb[:], scale=1.0)
            rstd = small.tile([8, 1], F32, tag="rstd")
            nc.vector.reciprocal(rstd[:], srt[:])
            gv2 = small.tile([8, 2], F32, tag="gv2")
            nc.vector.tensor_copy(gv2[:, 0:1], rstd[:])
            nc.vector.tensor_copy(gv2[:, 1:2], gst[:, 0:1])
            bc_ps = psB.tile([128, 2], F32, tag="att", name=f"bc_{t}")
            nc.tensor.matmul(bc_ps[:], lhsT=GT_sb[:], rhs=gv2[:],
                             start=True, stop=True)
            sc = small.tile([128, CT, 2], F32, tag="sc")
            nc.vector.tensor_mul(sc[:, t, 0:1], bc_ps[:, 0:1], gam_col(t))
            nc.vector.tensor_mul(sc[:, t, 1:2], bc_ps[:, 1:2], sc[:, t, 0:1])
            nc.vector.tensor_sub(sc[:, t, 1:2], bet_col(t), sc[:, t, 1:2])
            # apply per half, Vector + GpSimd in parallel
            for n, eng in ((0, nc.vector), (1, nc.gpsimd)):
                eng.tensor_scalar(
                    out=h_sb[:, t, n * 512:(n + 1) * 512],
                    in0=x_sb[:, t, n * 512:(n + 1) * 512],
                    scalar1=sc[:, t, 0:1], scalar2=sc[:, t, 1:2],
                    op0=mybir.AluOpType.mult, op1=mybir.AluOpType.add)
        # preload ACT exp table after the last sqrt (forced order so Tile
        # can't hoist it between the sqrts and thrash the table RAM)
        dummy3 = small.tile([1, 1], F32, tag="dummy3")
        expd = nc.scalar.activation(dummy3[:], dummy[:],
                                    mybir.ActivationFunctionType.Exp, scale=1.0)
        bass_rust.add_dep_helper(expd.ins, last_sqrt.ins, reason="ACT table order")

        qk_sb = qkp.tile([128, 2 * CT, HW], BF16)
        v_sb = vpl.tile([128, ST, NH * 66], FP8E4)
        nc.vector.memset(
            v_sb[:].rearrange("p m (h e) -> p m h e", e=66)[:, :, :, 64], 1.0)
        h2_sb = h2p.tile([128, CT, HW], BF16)

        if FP8_CONV:
            def emit_qk_conv(m):
                ps = psB.tile([128, HW], F32, tag="att", name=f"qkps{m}")
                for kk in (0, 2):
                    for n in range(2):
                        nc.tensor.matmul(
                            ps[:, n * 512:(n + 1) * 512],
                            lhsT=wq_sb[:, kk:kk + 2, m * 128:(m + 1) * 128],
                            rhs=h_sb[:, kk:kk + 2, n * 512:(n + 1) * 512],
                            start=(kk == 0), stop=(kk == 2),
                            perf_mode=mybir.MatmulPerfMode.DoubleRow)
                nc.scalar.add(qk_sb[:, m, :], ps[:], add=qkb_col(m))

            def emit_v_conv(m):
                psv = psB.tile([128, 512], F32, tag="att", name=f"vps{m}")
                for kk in (0, 2):
                    nc.tensor.matmul(
                        psv[:],
                        lhsT=h_sb[:, kk:kk + 2, m * 128:(m + 1) * 128],
                        rhs=wq_sb[:, kk:kk + 2, 2 * C:3 * C],
                        start=(kk == 0), stop=False,
                        perf_mode=mybir.MatmulPerfMode.DoubleRow)
                # rank-1 v-bias add: psv += ones[s] * vb[vc]
                nc.tensor.matmul(psv[:], lhsT=ones1[:], rhs=vb_sb[:],
                                 start=False, stop=True)
                nc.vector.tensor_copy(
                    v_sb[:, m, :].rearrange("p (h e) -> p h e", e=66)[:, :, 0:64],
                    psv[:].rearrange("p (h d) -> p h d", d=64))
        else:
            def emit_qk_conv(m):
                ps = psB.tile([128, HW], F32, tag="att", name=f"qkps{m}")
                for k in range(CT):
                    for n in range(2):
                        nc.tensor.matmul(
                            ps[:, n * 512:(n + 1) * 512],
                            lhsT=wq_sb[:, k, m * 128:(m + 1) * 128],
                            rhs=h_sb[:, k, n * 512:(n + 1) * 512],
                            start=(k == 0), stop=(k == CT - 1))
                nc.scalar.add(qk_sb[:, m, :], ps[:], add=qkb_col(m))

            def emit_v_conv(m):
                psv = psB.tile([128, 512], F32, tag="att", name=f"vps{m}")
                for k in range(CT):
                    nc.tensor.matmul(
                        psv[:],
                        lhsT=h_sb[:, k, m * 128:(m + 1) * 128],
                        rhs=wq_sb[:, k, 2 * C:3 * C],
                        start=(k == 0), stop=False)
                nc.tensor.matmul(psv[:], lhsT=ones1[:], rhs=vb_sb[:],
                                 start=False, stop=True)
                nc.vector.tensor_copy(
                    v_sb[:, m, :].rearrange("p (h e) -> p h e", e=66)[:, :, 0:64],
                    psv[:].rearrange("p (h d) -> p h d", d=64))

        # q/k tiles for pair 0 first, so its scores can start immediately
        emit_qk_conv(0)
        emit_qk_conv(4)
        # remaining conv work, interleaved into pair 0's attnv slot below
        conv_work = [lambda m=m: emit_qk_conv(m) for m in (1, 5, 2, 6, 3, 7)]
        conv_work += [lambda m=m: emit_v_conv(m) for m in range(ST)]

        def emit_scores_step(cur, step):
            pss = []
            for (h, pt) in cur:
                base = 64 * (h % 2)
                ps = psA.tile([128, HW], F32, tag="sc", name=f"scps{h}_{step}")
                pss.append(ps)
                kT = qk_sb[base:base + 64, CT + h // 2,
                           step * 128:(step + 1) * 128]
                qT = qk_sb[base:base + 64, h // 2, :]
                for n in range(2):
                    nc.tensor.matmul(
                        ps[:, n * 512:(n + 1) * 512], lhsT=kT,
                        rhs=qT[:, n * 512:(n + 1) * 512],
                        start=True, stop=True)
            (hA, ptA), (hB, ptB) = cur
            expi = nc.scalar.activation(
                ptA[:, step, :], pss[0][:],
                mybir.ActivationFunctionType.Exp,
                scale=float(DH ** -0.5))
            if hA == 0 and step == 0:
                first_exp.append(expi)
            nc.vector._custom_dve(
                EXP64, out=ptB[:, step, :], in0=pss[1][:],
                s0=float(DH ** -0.5) / 64.0)

        def emit_attnv_sm(h, pt, sm, state, last=False):
            # DoubleRow packs a j-tile pair per matmul (fp8 weights
            # 2-per-cell, K=256 virtual)
            if sm == 0:
                state[h] = psB.tile([128, HW], F32, tag="att", name=f"po{h}")
            po = state[h]
            jj = 2 * sm
            v2_ = v_sb[:].rearrange(
                "p m (hh e) -> p m hh e", e=66)[:, jj:jj + 2, h, 0:65]
            for n in range(2):
                nc.tensor.matmul(
                    po[0:65, n * 512:(n + 1) * 512],
                    lhsT=v2_,
                    rhs=pt[:, jj:jj + 2, n * 512:(n + 1) * 512],
                    start=(sm == 0), stop=(sm == 3),
                    perf_mode=mybir.MatmulPerfMode.DoubleRow)
            if sm == 3:
                o2t = o2tp.tile([80, HW], BF16, tag="o2t")
                if h % 2 == 0:
                    nc.scalar.copy(o2t[0:65, :], po[0:65, :])
                else:
                    nc.vector.tensor_copy(o2t[0:65, :], po[0:65, :])
                o2tr = o2trp.tile([128, ST, 80], BF16, tag="o2tr")
                nc.sync.dma_start_transpose(o2tr[:], o2t[:])
                linv = small.tile([128, ST], F32, tag="linv")
                nc.vector.reciprocal(linv[:], o2tr[:, :, 64])
                o2 = o2p.tile([128, 512], BF16, tag="o2")
                lap = linv[:]
                lbc = bass.AP(tensor=lap.tensor, offset=lap.offset,
                              ap=[[lap.ap[0][0], 128], [1, ST], [0, 64]])
                nc.gpsimd.tensor_mul(
                    o2[:].rearrange("p (q d) -> p q d", d=64),
                    o2tr[:, :, 0:64], lbc)
                # alternate h2 round-trip DMA queues by head parity so the
                # two heads of a pair don't serialize on one DGE queue; the
                # round-trip goes in halves so the readback pipelines with
                # the write
                dmae = nc.sync if (h % 2 == 0) != last else nc.gpsimd
                k, half = h // 2, h % 2
                h2f = h2_d[:].rearrange("c s -> (c s)")
                o2q = o2[:].rearrange("p (q d) -> p q d", d=64)
                wrs = []
                for qh in range(2):
                    wrs.append(dmae.dma_start(
                        out=h2f[h * 65536 + qh * 32768:
                                h * 65536 + (qh + 1) * 32768]
                        .rearrange("(q p d) -> p q d", p=128, d=64),
                        in_=o2q[:, qh * 4:(qh + 1) * 4, :]))
                for qh in range(2):
                    rd = dmae.dma_start(
                        out=h2_sb[64 * half + 32 * qh:64 * half + 32 * qh + 32,
                                  k, :],
                        in_=h2_d[h * 64 + 32 * qh:h * 64 + 32 * qh + 32, :])
                    bass_rust.add_dep_helper(rd.ins, wrs[qh].ins,
                                             reason="h2 RAW")

        proj_pp = {}

        def emit_proj(o, ks, finish, pool=None, tag="sc"):
            if o not in proj_pp:
                proj_pp[o] = (pool or psA).tile([128, HW], F32, tag=tag,
                                                name=f"pp{o}")
            pp = proj_pp[o]
            for k in ks:
                for n in range(2):
                    nc.tensor.matmul(
                        pp[:, n * 512:(n + 1) * 512],
                        lhsT=wp_sb[:, k, o * 128:(o + 1) * 128],
                        rhs=h2_sb[:, k, n * 512:(n + 1) * 512],
                        start=(k == 0), stop=(k == CT - 1))
            if finish:
                # out = (proj + proj_b) + x  in one fused DVE op
                ot = outp.tile([128, HW], BF16, tag="ot")
                nc.vector.scalar_tensor_tensor(
                    out=ot[:], in0=pp[:], scalar=pb_col(o),
                    in1=x_sb[:, o, :],
                    op0=mybir.AluOpType.add, op1=mybir.AluOpType.add)
                eng = nc.sync if o % 2 == 0 else nc.scalar
                eng.dma_start(out=out_d[o * 128:(o + 1) * 128, :], in_=ot[:])
                del proj_pp[o]

        # ---- attention pair loop (software pipelined, pairs 0-3) ----
        first_exp = []
        prev = None
        for hp_i in range(4):
            hA, hB = 2 * hp_i, 2 * hp_i + 1
            ptA = ptp.tile([128, ST, HW], FP8E5, tag="pt", name=f"pt{hA}")
            ptB = ptp.tile([128, ST, HW], FP8E5, tag="pt", name=f"pt{hB}")
            cur = [(hA, ptA), (hB, ptB)]
            state = {}
            for step in range(8):
                emit_scores_step(cur, step)
                if prev is not None:
                    h, pt = prev[step // 4]
                    emit_attnv_sm(h, pt, step % 4, state)
                elif conv_work:
                    # pair 0: fill the attnv slot with remaining conv tiles
                    conv_work.pop(0)()
                    if conv_work and step % 2 == 1:
                        conv_work.pop(0)()
            while prev is None and conv_work:
                conv_work.pop(0)()
            if hp_i == 0:
                # proj weights DMA deferred past the input-load window (wp
                # isn't needed until the drain); dep stops Tile hoisting it
                wpdma = nc.gpsimd.dma_start(
                    out=wp_sb[:],
                    in_=wp_d[:].rearrange("(t p) o -> p t o", p=128))
                bass_rust.add_dep_helper(wpdma.ins, first_exp[0].ins,
                                         reason="defer wp load")
            prev = cur

        # ---- drain: last pair's attnv at double rate, proj spread under
        # the h2 round-trip latency so the PE stays warm ----
        (h6, pt6), (h7, pt7) = prev
        state = {}
        for sm in range(4):
            emit_attnv_sm(h6, pt6, sm, state, last=True)
            emit_attnv_sm(h7, pt7, sm, state, last=True)
            if sm == 1:
                emit_proj(0, [0], finish=False)
            if sm == 2:
                emit_proj(1, [0], finish=False)
        emit_proj(0, [1], finish=False)
        emit_proj(1, [1], finish=False)
        emit_proj(0, [2], finish=False)
        emit_proj(1, [2], finish=False)
        emit_proj(2, [0], finish=False, pool=psB, tag="att")
        emit_proj(2, [1], finish=False, pool=psB, tag="att")
        # the rest is emitted after the sm3 normalize chains above, so these
        # matmuls execute during the h2 DRAM round-trip and keep the PE warm
        emit_proj(3, [0], finish=False, pool=psB, tag="att")
        emit_proj(2, [2], finish=False, pool=psB, tag="att")
        emit_proj(3, [1], finish=False, pool=psB, tag="att")
        emit_proj(3, [2], finish=False, pool=psB, tag="att")

        # ---- proj finish ----
        emit_proj(0, [3], finish=True)
        emit_proj(1, [3], finish=True)
        emit_proj(2, [3], finish=True)
        emit_proj(3, [3], finish=True)

    nc.compile()
    return nc


def _host_prep(x, norm_gamma, norm_beta, qkv_w, qkv_b, proj_w, proj_b):
    x = np.asarray(x, dtype=np.float32).reshape(B, C, HW)
    qkv_w = np.asarray(qkv_w, dtype=np.float32)
    qkv_b = np.asarray(qkv_b, dtype=np.float32)
    proj_w = np.asarray(proj_w, dtype=np.float32)
    proj_b = np.asarray(proj_b, dtype=np.float32)

    wq_np = np.ascontiguousarray(qkv_w.T)
    if FP8_CONV:
        wqkvT = np.clip(wq_np, -440.0, 440.0).astype(ml_dtypes.float8_e4m3fn)
    else:
        wqkvT = wq_np.astype(ml_dtypes.bfloat16)
    wprojT = np.ascontiguousarray(proj_w.T).astype(ml_dtypes.bfloat16)
    qkb = np.ascontiguousarray(qkv_b[:2 * C])
    vb = np.ascontiguousarray(qkv_b[2 * C:]).astype(ml_dtypes.bfloat16)
    pb = np.ascontiguousarray(proj_b, dtype=np.float32)

    G = np.zeros((128, GPT), np.float32)
    for p in range(128):
        G[p, p // CPG] = 1.0 / CPG
    GT = np.zeros((8, 128), np.float32)
    for p in range(128):
        GT[p // CPG, p] = 1.0

    gamma = np.asarray(norm_gamma, dtype=np.float32)
    beta = np.asarray(norm_beta, dtype=np.float32)
    # packed per-partition constants [128, 28]:
    # cols 0-3 gamma, 4-7 beta, 8-15 qkb, 16-19 proj_b, 20-27 G
    cpack = np.zeros((128, 28), np.float32)
    cpack[:, 0:4] = gamma.reshape(CT, 128).T
    cpack[:, 4:8] = beta.reshape(CT, 128).T
    cpack[:, 8:16] = qkb.reshape(2 * CT, 128).T
    cpack[:, 16:20] = pb.reshape(CT, 128).T
    cpack[:, 20:28] = G
    cpack = np.ascontiguousarray(cpack)

    in_maps = []
    for b in range(B):
        in_maps.append({
            "x": np.ascontiguousarray(x[b]).astype(ml_dtypes.bfloat16),
            "wqkvT": wqkvT, "wprojT": wprojT,
            "cpack": cpack, "vb": vb, "GT": GT,
        })
    return in_maps


def _run(inputs: dict, trace: bool = False, tmpdir=None):
    if "nc" not in _CACHE:
        _CACHE["nc"] = _build()
    nc = _CACHE["nc"]
    in_maps = _host_prep(**inputs)
    res = run_bass_kernel_spmd(nc, in_maps, core_ids=list(range(8)), trace=trace,
                               tmpdir=tmpdir)
    out = np.stack([np.asarray(r["out"]).astype(np.float32)
                    for r in res.results]).reshape(B, C, 32, 32)
    return out, res


def kernel(**inputs):
    out, _ = _run(inputs, trace=False)
    return out


# revision 30
# speedup vs baseline: 1.0247x; 1.0085x over previous
"""Trainium2 Bass kernel for nn_AttentionBlock (B=8, C=512, H=W=32, NH=8, DH=64).

Sharding: pure data-parallel — one batch element per NeuronCore (8 cores).
Per-core pipeline (channels-on-partitions layout, HW=1024 spatial):
  groupnorm -> qkv 1x1conv (fp8 DoubleRow matmul) -> attention:
    scores computed transposed (pT[j,i] = exp(k_j . q_i / 8), exp split
    between ScalarE ACT and a custom DVE EXP64 op, no softmax reductions),
    then out2T[d,i] = V^T-stationary matmul streaming pT, row sums via a
    ones-column in V, transpose via DMA xbar, normalize on GpSimd
    -> reshape via DRAM round-trip -> proj 1x1conv (bf16) -> residual.
Software-pipelined over head pairs; conv tiles fill pair 0's attnv slot,
proj partials fill the drain slot.

v2 changes vs v1 (148.9us):
  - x DMA split per half-tile + groupnorm starts per-tile on arrival
  - qkv convs in fp8e4 DoubleRow (half the matmul count)
  - qk bias add + attn-out PSUM->SBUF cast moved to ScalarE (ACT identity/copy)
  - o2 normalize + xpb residual-bias adds moved to GpSimd
  - ACT table loads forced to (sqrt, exp) order once each
  - scores matmuls head-major for LDWEIGHTS adjacency
"""

import numpy as np
import ml_dtypes

import concourse.bass as bass
import concourse.mybir as mybir
import concourse.tile as tile
from concourse import bacc
from concourse.bass_utils import run_bass_kernel_spmd

F32 = mybir.dt.float32
BF16 = mybir.dt.bfloat16
FP8E4 = mybir.dt.float8e4
FP8E5 = mybir.dt.float8e5

B, C, HW = 8, 512, 1024
NH, DH = 8, 64
GROUPS, EPS = 32, 1e-5
CT = C // 128          # 4 channel tiles
ST = HW // 128         # 8 spatial tiles
GPT = 8                # groups per 128-channel tile
CPG = 16               # channels per group

FP8_CONV = True        # qkv conv in fp8e4 DoubleRow (proj stays bf16)

_CACHE: dict = {}


def _register_exp64():
    """Register the custom DVE op  out = (1 + in0*s0)^64  (approx exp(in0*s0*64)).

    1 mult + 1 add + 6 squarings = 8 ALU stages (the DVE datapath limit).
    The uop table is generated per-NEFF at compile time, no firmware change.
    """
    from concourse import dve_ops as DO
    if "EXP64_ANT" in DO._SUB_OPCODE_FOR_NAME:
        return next(op for op in DO.OPS if op.name == "EXP64_ANT")
    from concourse.dve_spec import Spec, Src0, C0, One, sq, lower, _has_src1
    from concourse.dve_uop import DveOpSpec

    body = sq(sq(sq(sq(sq(sq(One + Src0 * C0))))))
    spec = Spec(
        body=body,
        reference=lambda in0, in1, s0, s1, imm2:
            (1.0 + in0.astype(np.float32) * np.float32(s0)) ** 64,
    )
    row = max(DO._SUB_OPCODE_FOR_NAME.values()) + 1
    shas = {}
    for ver in ("v3", "v4"):
        try:
            u = lower(spec, ver=ver)
            shas[ver] = DveOpSpec(
                name="EXP64_ANT", opcode=row, uops=u, rd1_en=_has_src1(spec)
            ).sha(ver)
        except Exception:
            pass
    op = DO.DveOp("EXP64_ANT", spec, subdim=False, uops_sha=shas)
    DO.OPS.append(op)
    DO._SUB_OPCODE_FOR_NAME["EXP64_ANT"] = row
    DO.CUSTOM_DVE_SPECS["EXP64_ANT"] = spec
    return op


def _build():
    EXP64 = _register_exp64()
    nc = bacc.Bacc("TRN2", target_bir_lowering=False, debug=False, num_devices=8)

    WQDT = FP8E4 if FP8_CONV else BF16
    x_d = nc.declare_dram_parameter("x", [C, HW], BF16, isOutput=False)
    wq_d = nc.declare_dram_parameter("wqkvT", [C, 3 * C], WQDT, isOutput=False)
    wp_d = nc.declare_dram_parameter("wprojT", [C, C], BF16, isOutput=False)
    # all small per-partition constants packed into one [128, 28] array:
    # cols 0-3 gamma, 4-7 beta, 8-15 qkb, 16-19 proj_b, 20-27 G
    cpk_d = nc.declare_dram_parameter("cpack", [128, 28], F32, isOutput=False)
    vb_d = nc.declare_dram_parameter("vb", [C], BF16, isOutput=False)
    GT_d = nc.declare_dram_parameter("GT", [8, 128], F32, isOutput=False)
    out_d = nc.declare_dram_parameter("out", [C, HW], BF16, isOutput=True)
    h2_d = nc.dram_tensor("h2d", [C, HW], BF16)

    import bass_rust
    from contextlib import ExitStack

    with tile.TileContext(nc) as tc, ExitStack() as ctx:
        const = ctx.enter_context(tc.tile_pool(name="const", bufs=1))
        small = ctx.enter_context(tc.tile_pool(name="small", bufs=2))
        xp = ctx.enter_context(tc.tile_pool(name="xp", bufs=1))
        hp = ctx.enter_context(tc.tile_pool(name="hp", bufs=1))
        wqp = ctx.enter_context(tc.tile_pool(name="wqp", bufs=1))
        wpp = ctx.enter_context(tc.tile_pool(name="wpp", bufs=1))
        qkp = ctx.enter_context(tc.tile_pool(name="qkp", bufs=1))
        vpl = ctx.enter_context(tc.tile_pool(name="vpl", bufs=1))
        ptp = ctx.enter_context(tc.tile_pool(name="ptp", bufs=4))
        o2tp = ctx.enter_context(tc.tile_pool(name="o2tp", bufs=2))
        o2trp = ctx.enter_context(tc.tile_pool(name="o2trp", bufs=2))
        o2p = ctx.enter_context(tc.tile_pool(name="o2p", bufs=2))
        h2p = ctx.enter_context(tc.tile_pool(name="h2p", bufs=1))
        outp = ctx.enter_context(tc.tile_pool(name="outp", bufs=4))
        # PSUM: psA = scores (+ proj at drain), psB = convs/attnv; 8 banks
        psA = ctx.enter_context(tc.tile_pool(name="psA", bufs=2, space="PSUM"))
        psB = ctx.enter_context(tc.tile_pool(name="psB", bufs=2, space="PSUM"))

        dummy = small.tile([1, 1], F32, tag="dummy")
        nc.gpsimd.memset(dummy[:], 1.0)

        # ACT table preloads: sqrt now, exp forced after the last sqrt
        dummy2 = small.tile([1, 1], F32, tag="dummy2")
        nc.scalar.activation(dummy2[:], dummy[:],
                             mybir.ActivationFunctionType.Sqrt, bias=0.0, scale=1.0)

        cpk_sb = const.tile([128, 28], F32, tag="cpk")
        nc.scalar.dma_start(out=cpk_sb[:], in_=cpk_d[:])
        gam_col = lambda t: cpk_sb[:, 0 + t:1 + t]
        bet_col = lambda t: cpk_sb[:, 4 + t:5 + t]
        qkb_col = lambda m: cpk_sb[:, 8 + m:9 + m]
        pb_col = lambda o: cpk_sb[:, 16 + o:17 + o]
        GT_sb = const.tile([8, 128], F32, tag="GT")
        nc.scalar.dma_start(out=GT_sb[:], in_=GT_d[:])
        vb_sb = const.tile([1, C], BF16, tag="vb")
        nc.scalar.dma_start(out=vb_sb[:], in_=vb_d[:].rearrange("c -> () c"))
        ones1 = const.tile([1, 128], BF16, tag="ones1")
        nc.vector.memset(ones1[:], 1.0)
        zeros1 = const.tile([1, 128], BF16, tag="zeros1")
        nc.vector.memset(zeros1[:], 0.0)

        # ---- bulk input DMAs: full x tiles (4KB lines) spread over all
        # three DMA-capable engines; wq after; wp deferred to mid-kernel ----
        x_sb = xp.tile([128, CT, HW], BF16)
        x_r = x_d[:].rearrange("(t p) s -> t p s", p=128)
        x_eng = [nc.sync, nc.scalar, nc.gpsimd, nc.sync]
        for t in range(CT):
            x_eng[t].dma_start(out=x_sb[:, t, :], in_=x_r[t])

        wq_sb = wqp.tile([128, CT, 3 * C], WQDT)
        wq_r = wq_d[:].rearrange("(t p) o -> t p o", p=128)
        for k in range(CT):
            nc.gpsimd.dma_start(out=wq_sb[:, k, :], in_=wq_r[k])
        wp_sb = wpp.tile([128, CT, C], BF16)

        # ---- per-tile groupnorm (starts as each x tile arrives) ----
        eps_sb = small.tile([8, 1], F32, tag="eps")
        nc.vector.memset(eps_sb[:], float(EPS))
        HDT = FP8E4 if FP8_CONV else BF16
        h_sb = hp.tile([128, CT, HW], HDT)
        mv = small.tile([128, CT, 3], F32, tag="mv")
        last_sqrt = None
        for t in range(CT):
            st = small.tile([128, 2, 6], F32, tag="bnst")
            x3 = x_sb[:, t, :].rearrange("p (a f) -> p a f", a=2)
            nc.vector.bn_stats(st[:, 0, :], x3[:, 0, :])
            nc.vector.bn_stats(st[:, 1, :], x3[:, 1, :])
            nc.vector.bn_aggr(mv[:, t, 0:2], st[:])
            nc.vector.tensor_mul(mv[:, t, 2:3], mv[:, t, 0:1], mv[:, t, 0:1])
            psg = psB.tile([8, 3], F32, tag="att", name=f"g_{t}")
            nc.tensor.matmul(psg[:], lhsT=cpk_sb[:, 20:28], rhs=mv[:, t, :],
                             start=True, stop=True)
            gst = small.tile([8, 3], F32, tag="gst")
            nc.vector.tensor_copy(gst[:], psg[:])
            sqv = small.tile([8, 2], F32, tag="sqv")
            nc.vector.tensor_mul(sqv[:, 0:1], gst[:, 0:1], gst[:, 0:1])
            nc.vector.tensor_add(sqv[:, 1:2], gst[:, 1:2], gst[:, 2:3])
            nc.vector.tensor_sub(sqv[:, 1:2], sqv[:, 1:2], sqv[:, 0:1])
            srt = small.tile([8, 1], F32, tag="srt")
            last_sqrt = nc.scalar.activation(
                srt[:], sqv[:, 1:2], mybir.ActivationFunctionType.Sqrt,
                bias=eps_sb[:], scale=1.0)
            rstd = small.tile([8, 1], F32, tag="rstd")
            nc.vector.reciprocal(rstd[:], srt[:])
            gv2 = small.tile([8, 2], F32, tag="gv2")
            nc.vector.tensor_copy(gv2[:, 0:1], rstd[:])
            nc.vector.tensor_copy(gv2[:, 1:2], gst[:, 0:1])
            bc_ps = psB.tile([128, 2], F32, tag="att", name=f"bc_{t}")
            nc.tensor.matmul(bc_ps[:], lhsT=GT_sb[:], rhs=gv2[:],
                             start=True, stop=True)
            sc = small.tile([128, CT, 2], F32, tag="sc")
            nc.vector.tensor_mul(sc[:, t, 0:1], bc_ps[:, 0:1], gam_col(t))
            nc.vector.tensor_mul(sc[:, t, 1:2], bc_ps[:, 1:2], sc[:, t, 0:1])
            nc.vector.tensor_sub(sc[:, t, 1:2], bet_col(t), sc[:, t, 1:2])
            # apply per half, Vector + GpSimd in parallel
            for n, eng in ((0, nc.vector), (1, nc.gpsimd)):
                eng.tensor_scalar(
                    out=h_sb[:, t, n * 512:(n + 1) * 512],
                    in0=x_sb[:, t, n * 512:(n + 1) * 512],
                    scalar1=sc[:, t, 0:1], scalar2=sc[:, t, 1:2],
                    op0=mybir.AluOpType.mult, op1=mybir.AluOpType.add)
        # preload ACT exp table after the last sqrt (forced order so Tile
        # can't hoist it between the sqrts and thrash the table RAM)
        dummy3 = small.tile([1, 1], F32, tag="dummy3")
        expd = nc.scalar.activation(dummy3[:], dummy[:],
                                    mybir.ActivationFunctionType.Exp, scale=1.0)
        bass_rust.add_dep_helper(expd.ins, last_sqrt.ins, reason="ACT table order")

        qk_sb = qkp.tile([128, 2 * CT, HW], BF16)
        v_sb = vpl.tile([128, ST, NH * 66], FP8E4)
        nc.vector.memset(
            v_sb[:].rearrange("p m (h e) -> p m h e", e=66)[:, :, :, 64], 1.0)
        h2_sb = h2p.tile([128, CT, HW], BF16)

        if FP8_CONV:
            def emit_qk_conv(m):
                ps = psB.tile([128, HW], F32, tag="att", name=f"qkps{m}")
                for kk in (0, 2):
                    for n in range(2):
                        nc.tensor.matmul(
                            ps[:, n * 512:(n + 1) * 512],
                            lhsT=wq_sb[:, kk:kk + 2, m * 128:(m + 1) * 128],
                            rhs=h_sb[:, kk:kk + 2, n * 512:(n + 1) * 512],
                            start=(kk == 0), stop=(kk == 2),
                            perf_mode=mybir.MatmulPerfMode.DoubleRow)
                nc.scalar.add(qk_sb[:, m, :], ps[:], add=qkb_col(m))

            def emit_v_conv(m):
                psv = psB.tile([128, 512], F32, tag="att", name=f"vps{m}")
                for kk in (0, 2):
                    nc.tensor.matmul(
                        psv[:],
                        lhsT=h_sb[:, kk:kk + 2, m * 128:(m + 1) * 128],
                        rhs=wq_sb[:, kk:kk + 2, 2 * C:3 * C],
                        start=(kk == 0), stop=False,
                        perf_mode=mybir.MatmulPerfMode.DoubleRow)
                # rank-1 v-bias add: psv += ones[s] * vb[vc]
                nc.tensor.matmul(psv[:], lhsT=ones1[:], rhs=vb_sb[:],
                                 start=False, stop=True)
                nc.vector.tensor_copy(
                    v_sb[:, m, :].rearrange("p (h e) -> p h e", e=66)[:, :, 0:64],
                    psv[:].rearrange("p (h d) -> p h d", d=64))
        else:
            def emit_qk_conv(m):
                ps = psB.tile([128, HW], F32, tag="att", name=f"qkps{m}")
                for k in range(CT):
                    for n in range(2):
                        nc.tensor.matmul(
                            ps[:, n * 512:(n + 1) * 512],
                            lhsT=wq_sb[:, k, m * 128:(m + 1) * 128],
                            rhs=h_sb[:, k, n * 512:(n + 1) * 512],
                            start=(k == 0), stop=(k == CT - 1))
                nc.scalar.add(qk_sb[:, m, :], ps[:], add=qkb_col(m))

            def emit_v_conv(m):
                psv = psB.tile([128, 512], F32, tag="att", name=f"vps{m}")
                for k in range(CT):
                    nc.tensor.matmul(
                        psv[:],
                        lhsT=h_sb[:, k, m * 128:(m + 1) * 128],
                        rhs=wq_sb[:, k, 2 * C:3 * C],
                        start=(k == 0), stop=False)
                nc.tensor.matmul(psv[:], lhsT=ones1[:], rhs=vb_sb[:],
                                 start=False, stop=True)
                nc.vector.tensor_copy(
                    v_sb[:, m, :].rearrange("p (h e) -> p h e", e=66)[:, :, 0:64],
                    psv[:].rearrange("p (h d) -> p h d", d=64))

        # q/k tiles for pair 0 first, so its scores can start immediately
        emit_qk_conv(0)
        emit_qk_conv(4)
        # remaining conv work, interleaved into pair 0's attnv slot below
        conv_work = [lambda m=m: emit_qk_conv(m) for m in (1, 5, 2, 6, 3, 7)]
        conv_work += [lambda m=m: emit_v_conv(m) for m in range(ST)]

        def emit_scores_step(cur, step):
            pss = []
            for (h, pt) in cur:
                base = 64 * (h % 2)
                ps = psA.tile([128, HW], F32, tag="sc", name=f"scps{h}_{step}")
                pss.append(ps)
                kT = qk_sb[base:base + 64, CT + h // 2,
                           step * 128:(step + 1) * 128]
                qT = qk_sb[base:base + 64, h // 2, :]
                for n in range(2):
                    nc.tensor.matmul(
                        ps[:, n * 512:(n + 1) * 512], lhsT=kT,
                        rhs=qT[:, n * 512:(n + 1) * 512],
                        start=True, stop=True)
            (hA, ptA), (hB, ptB) = cur
            expi = nc.scalar.activation(
                ptA[:, step, :], pss[0][:],
                mybir.ActivationFunctionType.Exp,
                scale=float(DH ** -0.5))
            if hA == 0 and step == 0:
                first_exp.append(expi)
            nc.vector._custom_dve(
                EXP64, out=ptB[:, step, :], in0=pss[1][:],
                s0=float(DH ** -0.5) / 64.0)

        def emit_attnv_sm(h, pt, sm, state, last=False):
            # DoubleRow packs a j-tile pair per matmul (fp8 weights
            # 2-per-cell, K=256 virtual)
            if sm == 0:
                state[h] = psB.tile([128, HW], F32, tag="att", name=f"po{h}")
            po = state[h]
            jj = 2 * sm
            v2_ = v_sb[:].rearrange(
                "p m (hh e) -> p m hh e", e=66)[:, jj:jj + 2, h, 0:65]
            for n in range(2):
                nc.tensor.matmul(
                    po[0:65, n * 512:(n + 1) * 512],
                    lhsT=v2_,
                    rhs=pt[:, jj:jj + 2, n * 512:(n + 1) * 512],
                    start=(sm == 0), stop=(sm == 3),
                    perf_mode=mybir.MatmulPerfMode.DoubleRow)
            if sm == 3:
                o2t = o2tp.tile([80, HW], BF16, tag="o2t")
                if h % 2 == 0:
                    nc.scalar.copy(o2t[0:65, :], po[0:65, :])
                else:
                    nc.vector.tensor_copy(o2t[0:65, :], po[0:65, :])
                o2tr = o2trp.tile([128, ST, 80], BF16, tag="o2tr")
                teng = nc.scalar if (last and h % 2 == 1) else nc.sync
                teng.dma_start_transpose(o2tr[:], o2t[:])
                linv = small.tile([128, ST], F32, tag="linv")
                nc.vector.reciprocal(linv[:], o2tr[:, :, 64])
                o2 = o2p.tile([128, 512], BF16, tag="o2")
                lap = linv[:]
                lbc = bass.AP(tensor=lap.tensor, offset=lap.offset,
                              ap=[[lap.ap[0][0], 128], [1, ST], [0, 64]])
                meng = nc.vector if last else nc.gpsimd
                meng.tensor_mul(
                    o2[:].rearrange("p (q d) -> p q d", d=64),
                    o2tr[:, :, 0:64], lbc)
                # alternate h2 round-trip DMA queues by head parity so the
                # two heads of a pair don't serialize on one DGE queue; the
                # round-trip goes in halves so the readback pipelines with
                # the write
                dmae = nc.sync if (h % 2 == 0) != last else nc.gpsimd
                k, half = h // 2, h % 2
                h2f = h2_d[:].rearrange("c s -> (c s)")
                o2q = o2[:].rearrange("p (q d) -> p q d", d=64)
                wrs = []
                for qh in range(2):
                    wrs.append(dmae.dma_start(
                        out=h2f[h * 65536 + qh * 32768:
                                h * 65536 + (qh + 1) * 32768]
                        .rearrange("(q p d) -> p q d", p=128, d=64),
                        in_=o2q[:, qh * 4:(qh + 1) * 4, :]))
                for qh in range(2):
                    rd = dmae.dma_start(
                        out=h2_sb[64 * half + 32 * qh:64 * half + 32 * qh + 32,
                                  k, :],
                        in_=h2_d[h * 64 + 32 * qh:h * 64 + 32 * qh + 32, :])
                    bass_rust.add_dep_helper(rd.ins, wrs[qh].ins,
                                             reason="h2 RAW")

        proj_pp = {}

        def emit_proj(o, ks, finish, pool=None, tag="sc"):
            if o not in proj_pp:
                proj_pp[o] = (pool or psA).tile([128, HW], F32, tag=tag,
                                                name=f"pp{o}")
            pp = proj_pp[o]
            for k in ks:
                for n in range(2):
                    nc.tensor.matmul(
                        pp[:, n * 512:(n + 1) * 512],
                        lhsT=wp_sb[:, k, o * 128:(o + 1) * 128],
                        rhs=h2_sb[:, k, n * 512:(n + 1) * 512],
                        start=(k == 0), stop=(k == CT - 1))
            if finish:
                ot = outp.tile([128, HW], BF16, tag="ot")
                if o < 2:
                    # out = (proj + proj_b) + x in one fused DVE op
                    nc.vector.scalar_tensor_tensor(
                        out=ot[:], in0=pp[:], scalar=pb_col(o),
                        in1=x_sb[:, o, :],
                        op0=mybir.AluOpType.add, op1=mybir.AluOpType.add)
                else:
                    # split: ScalarE adds proj_b (PSUM read), VectorE then
                    # does a cheap 2x-mode bf16 add of x — halves the
                    # serial VectorE time at the very end of the kernel
                    tb = outp.tile([128, HW], BF16, tag="tb")
                    nc.scalar.add(tb[:], pp[:], add=pb_col(o))
                    nc.vector.tensor_add(ot[:], tb[:], x_sb[:, o, :])
                eng = nc.sync if o % 2 == 0 else nc.scalar
                eng.dma_start(out=out_d[o * 128:(o + 1) * 128, :], in_=ot[:])
                del proj_pp[o]

        # ---- attention pair loop (software pipelined, pairs 0-3) ----
        first_exp = []
        astate = {}
        prev = None
        for hp_i in range(4):
            hA, hB = 2 * hp_i, 2 * hp_i + 1
            ptA = ptp.tile([128, ST, HW], FP8E5, tag="pt", name=f"pt{hA}")
            ptB = ptp.tile([128, ST, HW], FP8E5, tag="pt", name=f"pt{hB}")
            cur = [(hA, ptA), (hB, ptB)]
            state = {}
            for step in range(8):
                emit_scores_step(cur, step)
                if prev is not None:
                    h, pt = prev[step // 4]
                    emit_attnv_sm(h, pt, step % 4, astate)
                    if hp_i == 3 and step >= 5:
                        # head 6's attnv starts inside pair 3's own window
                        # (its pt j-tiles are ready two steps after their
                        # scores), shortening the drain
                        emit_attnv_sm(cur[0][0], cur[0][1], step - 5, astate,
                                      last=True)
                elif conv_work:
                    # pair 0: fill the attnv slot with remaining conv tiles
                    conv_work.pop(0)()
                    if conv_work and step % 2 == 1:
                        conv_work.pop(0)()
            while prev is None and conv_work:
                conv_work.pop(0)()
            if hp_i == 0:
                # proj weights DMA deferred past the input-load window (wp
                # isn't needed until the drain); dep stops Tile hoisting it
                wpdma = nc.gpsimd.dma_start(
                    out=wp_sb[:],
                    in_=wp_d[:].rearrange("(t p) o -> p t o", p=128))
                bass_rust.add_dep_helper(wpdma.ins, first_exp[0].ins,
                                         reason="defer wp load")
            prev = cur

        # ---- drain: finish head 6, head 7 at full rate, proj spread
        # under the h2 round-trip latency ----
        (h6, pt6), (h7, pt7) = prev
        emit_attnv_sm(h6, pt6, 3, astate, last=True)
        for sm in range(4):
            emit_attnv_sm(h7, pt7, sm, astate, last=True)
            if sm == 1:
                emit_proj(0, [0], finish=False)
            if sm == 2:
                emit_proj(1, [0], finish=False)
        emit_proj(0, [1], finish=False)
        emit_proj(1, [1], finish=False)
        emit_proj(0, [2], finish=False)
        emit_proj(1, [2], finish=False)
        emit_proj(2, [0], finish=False, pool=psB, tag="att")
        emit_proj(2, [1], finish=False, pool=psB, tag="att")
        # the rest is emitted after the sm3 normalize chains above, so these
        # matmuls execute during the h2 DRAM round-trip and keep the PE warm
        emit_proj(3, [0], finish=False, pool=psB, tag="att")
        emit_proj(2, [2], finish=False, pool=psB, tag="att")
        emit_proj(3, [1], finish=False, pool=psB, tag="att")
        emit_proj(3, [2], finish=False, pool=psB, tag="att")

        # ---- proj finish ----
        emit_proj(0, [3], finish=True)
        emit_proj(1, [3], finish=True)
        emit_proj(2, [3], finish=True)
        emit_proj(3, [3], finish=True)

    nc.compile()
    return nc


def _host_prep(x, norm_gamma, norm_beta, qkv_w, qkv_b, proj_w, proj_b):
    x = np.asarray(x, dtype=np.float32).reshape(B, C, HW)
    qkv_w = np.asarray(qkv_w, dtype=np.float32)
    qkv_b = np.asarray(qkv_b, dtype=np.float32)
    proj_w = np.asarray(proj_w, dtype=np.float32)
    proj_b = np.asarray(proj_b, dtype=np.float32)

    wq_np = np.ascontiguousarray(qkv_w.T)
    if FP8_CONV:
        wqkvT = np.clip(wq_np, -440.0, 440.0).astype(ml_dtypes.float8_e4m3fn)
    else:
        wqkvT = wq_np.astype(ml_dtypes.bfloat16)
    wprojT = np.ascontiguousarray(proj_w.T).astype(ml_dtypes.bfloat16)
    qkb = np.ascontiguousarray(qkv_b[:2 * C])
    vb = np.ascontiguousarray(qkv_b[2 * C:]).astype(ml_dtypes.bfloat16)
    pb = np.ascontiguousarray(proj_b, dtype=np.float32)

    G = np.zeros((128, GPT), np.float32)
    for p in range(128):
        G[p, p // CPG] = 1.0 / CPG
    GT = np.zeros((8, 128), np.float32)
    for p in range(128):
        GT[p // CPG, p] = 1.0

    gamma = np.asarray(norm_gamma, dtype=np.float32)
    beta = np.asarray(norm_beta, dtype=np.float32)
    # packed per-partition constants [128, 28]:
    # cols 0-3 gamma, 4-7 beta, 8-15 qkb, 16-19 proj_b, 20-27 G
    cpack = np.zeros((128, 28), np.float32)
    cpack[:, 0:4] = gamma.reshape(CT, 128).T
    cpack[:, 4:8] = beta.reshape(CT, 128).T
    cpack[:, 8:16] = qkb.reshape(2 * CT, 128).T
    cpack[:, 16:20] = pb.reshape(CT, 128).T
    cpack[:, 20:28] = G
    cpack = np.ascontiguousarray(cpack)

    in_maps = []
    for b in range(B):
        in_maps.append({
            "x": np.ascontiguousarray(x[b]).astype(ml_dtypes.bfloat16),
            "wqkvT": wqkvT, "wprojT": wprojT,
            "cpack": cpack, "vb": vb, "GT": GT,
        })
    return in_maps


def _run(inputs: dict, trace: bool = False, tmpdir=None):
    if "nc" not in _CACHE:
        _CACHE["nc"] = _build()
    nc = _CACHE["nc"]
    in_maps = _host_prep(**inputs)
    res = run_bass_kernel_spmd(nc, in_maps, core_ids=list(range(8)), trace=trace,
                               tmpdir=tmpdir)
    out = np.stack([np.asarray(r["out"]).astype(np.float32)
                    for r in res.results]).reshape(B, C, 32, 32)
    return out, res


def kernel(**inputs):
    out, _ = _run(inputs, trace=False)
    return out


# revision 31
# speedup vs baseline: 1.0445x; 1.0193x over previous
"""Trainium2 Bass kernel for nn_AttentionBlock (B=8, C=512, H=W=32, NH=8, DH=64).

Sharding: pure data-parallel — one batch element per NeuronCore (8 cores).
Per-core pipeline (channels-on-partitions layout, HW=1024 spatial):
  groupnorm -> qkv 1x1conv (fp8 DoubleRow matmul) -> attention:
    scores computed transposed (pT[j,i] = exp(k_j . q_i / 8), exp split
    between ScalarE ACT and a custom DVE EXP64 op, no softmax reductions),
    then out2T[d,i] = V^T-stationary matmul streaming pT, row sums via a
    ones-column in V, transpose via DMA xbar, normalize on GpSimd
    -> reshape via DRAM round-trip -> proj 1x1conv (bf16) -> residual.
Software-pipelined over head pairs; conv tiles fill pair 0's attnv slot,
proj partials fill the drain slot.

v2 changes vs v1 (148.9us):
  - x DMA split per half-tile + groupnorm starts per-tile on arrival
  - qkv convs in fp8e4 DoubleRow (half the matmul count)
  - qk bias add + attn-out PSUM->SBUF cast moved to ScalarE (ACT identity/copy)
  - o2 normalize + xpb residual-bias adds moved to GpSimd
  - ACT table loads forced to (sqrt, exp) order once each
  - scores matmuls head-major for LDWEIGHTS adjacency
"""

import numpy as np
import ml_dtypes

import concourse.bass as bass
import concourse.mybir as mybir
import concourse.tile as tile
from concourse import bacc
from concourse.bass_utils import run_bass_kernel_spmd

F32 = mybir.dt.float32
BF16 = mybir.dt.bfloat16
FP8E4 = mybir.dt.float8e4
FP8E5 = mybir.dt.float8e5

B, C, HW = 8, 512, 1024
NH, DH = 8, 64
GROUPS, EPS = 32, 1e-5
CT = C // 128          # 4 channel tiles
ST = HW // 128         # 8 spatial tiles
GPT = 8                # groups per 128-channel tile
CPG = 16               # channels per group

FP8_CONV = True        # qkv conv in fp8e4 DoubleRow (proj stays bf16)

_CACHE: dict = {}


def _register_exp64():
    """Register the custom DVE op  out = (1 + in0*s0)^64  (approx exp(in0*s0*64)).

    1 mult + 1 add + 6 squarings = 8 ALU stages (the DVE datapath limit).
    The uop table is generated per-NEFF at compile time, no firmware change.
    """
    from concourse import dve_ops as DO
    if "EXP64_ANT" in DO._SUB_OPCODE_FOR_NAME:
        return next(op for op in DO.OPS if op.name == "EXP64_ANT")
    from concourse.dve_spec import Spec, Src0, C0, One, sq, lower, _has_src1
    from concourse.dve_uop import DveOpSpec

    body = sq(sq(sq(sq(sq(sq(One + Src0 * C0))))))
    spec = Spec(
        body=body,
        reference=lambda in0, in1, s0, s1, imm2:
            (1.0 + in0.astype(np.float32) * np.float32(s0)) ** 64,
    )
    row = max(DO._SUB_OPCODE_FOR_NAME.values()) + 1
    shas = {}
    for ver in ("v3", "v4"):
        try:
            u = lower(spec, ver=ver)
            shas[ver] = DveOpSpec(
                name="EXP64_ANT", opcode=row, uops=u, rd1_en=_has_src1(spec)
            ).sha(ver)
        except Exception:
            pass
    op = DO.DveOp("EXP64_ANT", spec, subdim=False, uops_sha=shas)
    DO.OPS.append(op)
    DO._SUB_OPCODE_FOR_NAME["EXP64_ANT"] = row
    DO.CUSTOM_DVE_SPECS["EXP64_ANT"] = spec
    return op


def _build():
    EXP64 = _register_exp64()
    nc = bacc.Bacc("TRN2", target_bir_lowering=False, debug=False, num_devices=8)

    WQDT = FP8E4 if FP8_CONV else BF16
    x_d = nc.declare_dram_parameter("x", [C, HW], BF16, isOutput=False)
    wq_d = nc.declare_dram_parameter("wqkvT", [C, 3 * C], WQDT, isOutput=False)
    wp_d = nc.declare_dram_parameter("wprojT", [C, C], BF16, isOutput=False)
    # all small per-partition constants packed into one [128, 28] array:
    # cols 0-3 gamma, 4-7 beta, 8-15 qkb, 16-19 proj_b, 20-27 G
    cpk_d = nc.declare_dram_parameter("cpack", [128, 28], F32, isOutput=False)
    vb_d = nc.declare_dram_parameter("vb", [C], BF16, isOutput=False)
    GT_d = nc.declare_dram_parameter("GT", [8, 128], F32, isOutput=False)
    out_d = nc.declare_dram_parameter("out", [C, HW], BF16, isOutput=True)
    h2_d = nc.dram_tensor("h2d", [C, HW], BF16)

    import bass_rust
    from contextlib import ExitStack

    with tile.TileContext(nc) as tc, ExitStack() as ctx:
        const = ctx.enter_context(tc.tile_pool(name="const", bufs=1))
        small = ctx.enter_context(tc.tile_pool(name="small", bufs=2))
        xp = ctx.enter_context(tc.tile_pool(name="xp", bufs=1))
        hp = ctx.enter_context(tc.tile_pool(name="hp", bufs=1))
        wqp = ctx.enter_context(tc.tile_pool(name="wqp", bufs=1))
        wpp = ctx.enter_context(tc.tile_pool(name="wpp", bufs=1))
        qkp = ctx.enter_context(tc.tile_pool(name="qkp", bufs=1))
        vpl = ctx.enter_context(tc.tile_pool(name="vpl", bufs=1))
        ptp = ctx.enter_context(tc.tile_pool(name="ptp", bufs=4))
        o2tp = ctx.enter_context(tc.tile_pool(name="o2tp", bufs=2))
        o2trp = ctx.enter_context(tc.tile_pool(name="o2trp", bufs=2))
        o2p = ctx.enter_context(tc.tile_pool(name="o2p", bufs=2))
        h2p = ctx.enter_context(tc.tile_pool(name="h2p", bufs=1))
        outp = ctx.enter_context(tc.tile_pool(name="outp", bufs=4))
        # PSUM: psA = scores (+ proj at drain), psB = convs/attnv; 8 banks
        psA = ctx.enter_context(tc.tile_pool(name="psA", bufs=2, space="PSUM"))
        psB = ctx.enter_context(tc.tile_pool(name="psB", bufs=2, space="PSUM"))

        dummy = small.tile([1, 1], F32, tag="dummy")
        nc.gpsimd.memset(dummy[:], 1.0)

        # ACT table preloads: sqrt now, exp forced after the last sqrt
        dummy2 = small.tile([1, 1], F32, tag="dummy2")
        nc.scalar.activation(dummy2[:], dummy[:],
                             mybir.ActivationFunctionType.Sqrt, bias=0.0, scale=1.0)

        cpk_sb = const.tile([128, 28], F32, tag="cpk")
        nc.scalar.dma_start(out=cpk_sb[:], in_=cpk_d[:])
        gam_col = lambda t: cpk_sb[:, 0 + t:1 + t]
        bet_col = lambda t: cpk_sb[:, 4 + t:5 + t]
        qkb_col = lambda m: cpk_sb[:, 8 + m:9 + m]
        pb_col = lambda o: cpk_sb[:, 16 + o:17 + o]
        GT_sb = const.tile([8, 128], F32, tag="GT")
        nc.scalar.dma_start(out=GT_sb[:], in_=GT_d[:])
        vb_sb = const.tile([1, C], BF16, tag="vb")
        nc.scalar.dma_start(out=vb_sb[:], in_=vb_d[:].rearrange("c -> () c"))
        ones1 = const.tile([1, 128], BF16, tag="ones1")
        nc.vector.memset(ones1[:], 1.0)
        zeros1 = const.tile([1, 128], BF16, tag="zeros1")
        nc.vector.memset(zeros1[:], 0.0)

        # ---- bulk input DMAs: full x tiles (4KB lines) spread over all
        # three DMA-capable engines; wq after; wp deferred to mid-kernel ----
        x_sb = xp.tile([128, CT, HW], BF16)
        x_r = x_d[:].rearrange("(t p) s -> t p s", p=128)
        x_eng = [nc.sync, nc.scalar, nc.gpsimd, nc.sync]
        for t in range(CT):
            x_eng[t].dma_start(out=x_sb[:, t, :], in_=x_r[t])

        wq_sb = wqp.tile([128, CT, 3 * C], WQDT)
        wq_r = wq_d[:].rearrange("(t p) o -> t p o", p=128)
        for k in range(CT):
            nc.gpsimd.dma_start(out=wq_sb[:, k, :], in_=wq_r[k])
        wp_sb = wpp.tile([128, CT, C], BF16)

        # ---- per-tile groupnorm (starts as each x tile arrives) ----
        eps_sb = small.tile([8, 1], F32, tag="eps")
        nc.vector.memset(eps_sb[:], float(EPS))
        HDT = FP8E4 if FP8_CONV else BF16
        h_sb = hp.tile([128, CT, HW], HDT)
        mv = small.tile([128, CT, 3], F32, tag="mv")
        last_sqrt = None
        for t in range(CT):
            st = small.tile([128, 2, 6], F32, tag="bnst")
            x3 = x_sb[:, t, :].rearrange("p (a f) -> p a f", a=2)
            nc.vector.bn_stats(st[:, 0, :], x3[:, 0, :])
            nc.vector.bn_stats(st[:, 1, :], x3[:, 1, :])
            nc.vector.bn_aggr(mv[:, t, 0:2], st[:])
            nc.vector.tensor_mul(mv[:, t, 2:3], mv[:, t, 0:1], mv[:, t, 0:1])
            psg = psB.tile([8, 3], F32, tag="att", name=f"g_{t}")
            nc.tensor.matmul(psg[:], lhsT=cpk_sb[:, 20:28], rhs=mv[:, t, :],
                             start=True, stop=True)
            gst = small.tile([8, 3], F32, tag="gst")
            nc.vector.tensor_copy(gst[:], psg[:])
            sqv = small.tile([8, 2], F32, tag="sqv")
            nc.vector.tensor_mul(sqv[:, 0:1], gst[:, 0:1], gst[:, 0:1])
            nc.vector.tensor_add(sqv[:, 1:2], gst[:, 1:2], gst[:, 2:3])
            nc.vector.tensor_sub(sqv[:, 1:2], sqv[:, 1:2], sqv[:, 0:1])
            srt = small.tile([8, 1], F32, tag="srt")
            last_sqrt = nc.scalar.activation(
                srt[:], sqv[:, 1:2], mybir.ActivationFunctionType.Sqrt,
                bias=eps_sb[:], scale=1.0)
            rstd = small.tile([8, 1], F32, tag="rstd")
            nc.vector.reciprocal(rstd[:], srt[:])
            gv2 = small.tile([8, 2], F32, tag="gv2")
            nc.vector.tensor_copy(gv2[:, 0:1], rstd[:])
            nc.vector.tensor_copy(gv2[:, 1:2], gst[:, 0:1])
            bc_ps = psB.tile([128, 2], F32, tag="att", name=f"bc_{t}")
            nc.tensor.matmul(bc_ps[:], lhsT=GT_sb[:], rhs=gv2[:],
                             start=True, stop=True)
            sc = small.tile([128, CT, 2], F32, tag="sc")
            nc.vector.tensor_mul(sc[:, t, 0:1], bc_ps[:, 0:1], gam_col(t))
            nc.vector.tensor_mul(sc[:, t, 1:2], bc_ps[:, 1:2], sc[:, t, 0:1])
            nc.vector.tensor_sub(sc[:, t, 1:2], bet_col(t), sc[:, t, 1:2])
            # apply per half, Vector + GpSimd in parallel
            for n, eng in ((0, nc.vector), (1, nc.gpsimd)):
                eng.tensor_scalar(
                    out=h_sb[:, t, n * 512:(n + 1) * 512],
                    in0=x_sb[:, t, n * 512:(n + 1) * 512],
                    scalar1=sc[:, t, 0:1], scalar2=sc[:, t, 1:2],
                    op0=mybir.AluOpType.mult, op1=mybir.AluOpType.add)
        # preload ACT exp table after the last sqrt (forced order so Tile
        # can't hoist it between the sqrts and thrash the table RAM)
        dummy3 = small.tile([1, 1], F32, tag="dummy3")
        expd = nc.scalar.activation(dummy3[:], dummy[:],
                                    mybir.ActivationFunctionType.Exp, scale=1.0)
        bass_rust.add_dep_helper(expd.ins, last_sqrt.ins, reason="ACT table order")

        qk_sb = qkp.tile([128, 2 * CT, HW], BF16)
        v_sb = vpl.tile([128, ST, NH * 66], FP8E4)
        nc.vector.memset(
            v_sb[:].rearrange("p m (h e) -> p m h e", e=66)[:, :, :, 64], 1.0)
        h2_sb = h2p.tile([128, CT, HW], BF16)

        if FP8_CONV:
            def emit_qk_conv(m):
                ps = psB.tile([128, HW], F32, tag="att", name=f"qkps{m}")
                for kk in (0, 2):
                    for n in range(2):
                        nc.tensor.matmul(
                            ps[:, n * 512:(n + 1) * 512],
                            lhsT=wq_sb[:, kk:kk + 2, m * 128:(m + 1) * 128],
                            rhs=h_sb[:, kk:kk + 2, n * 512:(n + 1) * 512],
                            start=(kk == 0), stop=(kk == 2),
                            perf_mode=mybir.MatmulPerfMode.DoubleRow)
                nc.scalar.add(qk_sb[:, m, :], ps[:], add=qkb_col(m))

            def emit_v_conv(m):
                psv = psB.tile([128, 512], F32, tag="att", name=f"vps{m}")
                for kk in (0, 2):
                    nc.tensor.matmul(
                        psv[:],
                        lhsT=h_sb[:, kk:kk + 2, m * 128:(m + 1) * 128],
                        rhs=wq_sb[:, kk:kk + 2, 2 * C:3 * C],
                        start=(kk == 0), stop=False,
                        perf_mode=mybir.MatmulPerfMode.DoubleRow)
                # rank-1 v-bias add: psv += ones[s] * vb[vc]
                nc.tensor.matmul(psv[:], lhsT=ones1[:], rhs=vb_sb[:],
                                 start=False, stop=True)
                nc.vector.tensor_copy(
                    v_sb[:, m, :].rearrange("p (h e) -> p h e", e=66)[:, :, 0:64],
                    psv[:].rearrange("p (h d) -> p h d", d=64))
        else:
            def emit_qk_conv(m):
                ps = psB.tile([128, HW], F32, tag="att", name=f"qkps{m}")
                for k in range(CT):
                    for n in range(2):
                        nc.tensor.matmul(
                            ps[:, n * 512:(n + 1) * 512],
                            lhsT=wq_sb[:, k, m * 128:(m + 1) * 128],
                            rhs=h_sb[:, k, n * 512:(n + 1) * 512],
                            start=(k == 0), stop=(k == CT - 1))
                nc.scalar.add(qk_sb[:, m, :], ps[:], add=qkb_col(m))

            def emit_v_conv(m):
                psv = psB.tile([128, 512], F32, tag="att", name=f"vps{m}")
                for k in range(CT):
                    nc.tensor.matmul(
                        psv[:],
                        lhsT=h_sb[:, k, m * 128:(m + 1) * 128],
                        rhs=wq_sb[:, k, 2 * C:3 * C],
                        start=(k == 0), stop=False)
                nc.tensor.matmul(psv[:], lhsT=ones1[:], rhs=vb_sb[:],
                                 start=False, stop=True)
                nc.vector.tensor_copy(
                    v_sb[:, m, :].rearrange("p (h e) -> p h e", e=66)[:, :, 0:64],
                    psv[:].rearrange("p (h d) -> p h d", d=64))

        # q/k tiles for pair 0 first, so its scores can start immediately
        emit_qk_conv(0)
        emit_qk_conv(4)
        # remaining conv work, interleaved into pair 0's attnv slot below
        conv_work = [lambda m=m: emit_qk_conv(m) for m in (1, 5, 2, 6, 3, 7)]
        conv_work += [lambda m=m: emit_v_conv(m) for m in range(ST)]

        def emit_scores_step(cur, step):
            pss = []
            for (h, pt) in cur:
                base = 64 * (h % 2)
                ps = psA.tile([128, HW], F32, tag="sc", name=f"scps{h}_{step}")
                pss.append(ps)
                kT = qk_sb[base:base + 64, CT + h // 2,
                           step * 128:(step + 1) * 128]
                qT = qk_sb[base:base + 64, h // 2, :]
                for n in range(2):
                    nc.tensor.matmul(
                        ps[:, n * 512:(n + 1) * 512], lhsT=kT,
                        rhs=qT[:, n * 512:(n + 1) * 512],
                        start=True, stop=True)
            (hA, ptA), (hB, ptB) = cur
            expi = nc.scalar.activation(
                ptA[:, step, :], pss[0][:],
                mybir.ActivationFunctionType.Exp,
                scale=float(DH ** -0.5))
            if hA == 0 and step == 0:
                first_exp.append(expi)
            nc.vector._custom_dve(
                EXP64, out=ptB[:, step, :], in0=pss[1][:],
                s0=float(DH ** -0.5) / 64.0)

        def emit_attnv_sm(h, pt, sm, state, last=False):
            # DoubleRow packs a j-tile pair per matmul (fp8 weights
            # 2-per-cell, K=256 virtual)
            if sm == 0:
                state[h] = psB.tile([128, HW], F32, tag="att", name=f"po{h}")
            po = state[h]
            jj = 2 * sm
            v2_ = v_sb[:].rearrange(
                "p m (hh e) -> p m hh e", e=66)[:, jj:jj + 2, h, 0:65]
            for n in range(2):
                nc.tensor.matmul(
                    po[0:65, n * 512:(n + 1) * 512],
                    lhsT=v2_,
                    rhs=pt[:, jj:jj + 2, n * 512:(n + 1) * 512],
                    start=(sm == 0), stop=(sm == 3),
                    perf_mode=mybir.MatmulPerfMode.DoubleRow)
            if sm == 3:
                o2t = o2tp.tile([80, HW], BF16, tag="o2t")
                if h % 2 == 0:
                    nc.scalar.copy(o2t[0:65, :], po[0:65, :])
                else:
                    nc.vector.tensor_copy(o2t[0:65, :], po[0:65, :])
                o2tr = o2trp.tile([128, ST, 80], BF16, tag="o2tr")
                teng = nc.scalar if (last and h % 2 == 1) else nc.sync
                teng.dma_start_transpose(o2tr[:], o2t[:])
                linv = small.tile([128, ST], F32, tag="linv")
                nc.vector.reciprocal(linv[:], o2tr[:, :, 64])
                o2 = o2p.tile([128, 512], BF16, tag="o2")
                lap = linv[:]
                lbc = bass.AP(tensor=lap.tensor, offset=lap.offset,
                              ap=[[lap.ap[0][0], 128], [1, ST], [0, 64]])
                meng = nc.vector if last else nc.gpsimd
                meng.tensor_mul(
                    o2[:].rearrange("p (q d) -> p q d", d=64),
                    o2tr[:, :, 0:64], lbc)
                # alternate h2 round-trip DMA queues by head parity so the
                # two heads of a pair don't serialize on one DGE queue; the
                # round-trip goes in halves so the readback pipelines with
                # the write
                if last:
                    # drain: both heads on the fast HWDGE queues
                    dmae = nc.sync if h % 2 == 0 else nc.scalar
                else:
                    dmae = nc.sync if h % 2 == 0 else nc.gpsimd
                k, half = h // 2, h % 2
                h2f = h2_d[:].rearrange("c s -> (c s)")
                o2q = o2[:].rearrange("p (q d) -> p q d", d=64)
                wrs = []
                for qh in range(2):
                    wrs.append(dmae.dma_start(
                        out=h2f[h * 65536 + qh * 32768:
                                h * 65536 + (qh + 1) * 32768]
                        .rearrange("(q p d) -> p q d", p=128, d=64),
                        in_=o2q[:, qh * 4:(qh + 1) * 4, :]))
                for qh in range(2):
                    rd = dmae.dma_start(
                        out=h2_sb[64 * half + 32 * qh:64 * half + 32 * qh + 32,
                                  k, :],
                        in_=h2_d[h * 64 + 32 * qh:h * 64 + 32 * qh + 32, :])
                    bass_rust.add_dep_helper(rd.ins, wrs[qh].ins,
                                             reason="h2 RAW")

        proj_pp = {}

        def emit_proj(o, ks, finish, pool=None, tag="sc"):
            if o not in proj_pp:
                proj_pp[o] = (pool or psA).tile([128, HW], F32, tag=tag,
                                                name=f"pp{o}")
            pp = proj_pp[o]
            for k in ks:
                for n in range(2):
                    nc.tensor.matmul(
                        pp[:, n * 512:(n + 1) * 512],
                        lhsT=wp_sb[:, k, o * 128:(o + 1) * 128],
                        rhs=h2_sb[:, k, n * 512:(n + 1) * 512],
                        start=(k == 0), stop=(k == CT - 1))
            if finish:
                ot = outp.tile([128, HW], BF16, tag="ot")
                if o < 2:
                    # out = (proj + proj_b) + x in one fused DVE op
                    nc.vector.scalar_tensor_tensor(
                        out=ot[:], in0=pp[:], scalar=pb_col(o),
                        in1=x_sb[:, o, :],
                        op0=mybir.AluOpType.add, op1=mybir.AluOpType.add)
                else:
                    # split: ScalarE adds proj_b (PSUM read), VectorE then
                    # does a cheap 2x-mode bf16 add of x — halves the
                    # serial VectorE time at the very end of the kernel
                    tb = outp.tile([128, HW], BF16, tag="tb")
                    nc.scalar.add(tb[:], pp[:], add=pb_col(o))
                    nc.vector.tensor_add(ot[:], tb[:], x_sb[:, o, :])
                # each output tile's write split across both HWDGE queues
                nc.sync.dma_start(out=out_d[o * 128:(o + 1) * 128, 0:512],
                                  in_=ot[:, 0:512])
                nc.scalar.dma_start(out=out_d[o * 128:(o + 1) * 128, 512:1024],
                                    in_=ot[:, 512:1024])
                del proj_pp[o]

        # ---- attention pair loop (software pipelined, pairs 0-3) ----
        first_exp = []
        astate = {}
        prev = None
        for hp_i in range(4):
            hA, hB = 2 * hp_i, 2 * hp_i + 1
            ptA = ptp.tile([128, ST, HW], FP8E5, tag="pt", name=f"pt{hA}")
            ptB = ptp.tile([128, ST, HW], FP8E5, tag="pt", name=f"pt{hB}")
            cur = [(hA, ptA), (hB, ptB)]
            state = {}
            for step in range(8):
                emit_scores_step(cur, step)
                if prev is not None:
                    h, pt = prev[step // 4]
                    emit_attnv_sm(h, pt, step % 4, astate)
                    if hp_i == 3 and step >= 5:
                        # head 6's attnv starts inside pair 3's own window
                        # (its pt j-tiles are ready two steps after their
                        # scores), shortening the drain
                        emit_attnv_sm(cur[0][0], cur[0][1], step - 5, astate,
                                      last=True)
                elif conv_work:
                    # pair 0: fill the attnv slot with remaining conv tiles
                    conv_work.pop(0)()
                    if conv_work and step % 2 == 1:
                        conv_work.pop(0)()
            while prev is None and conv_work:
                conv_work.pop(0)()
            if hp_i == 0:
                # proj weights DMA deferred past the input-load window (wp
                # isn't needed until the drain); dep stops Tile hoisting it
                wpdma = nc.gpsimd.dma_start(
                    out=wp_sb[:],
                    in_=wp_d[:].rearrange("(t p) o -> p t o", p=128))
                bass_rust.add_dep_helper(wpdma.ins, first_exp[0].ins,
                                         reason="defer wp load")
            prev = cur

        # ---- drain: finish head 6, head 7 at full rate, proj spread
        # under the h2 round-trip latency ----
        (h6, pt6), (h7, pt7) = prev
        emit_attnv_sm(h6, pt6, 3, astate, last=True)
        for sm in range(4):
            emit_attnv_sm(h7, pt7, sm, astate, last=True)
            if sm == 1:
                emit_proj(0, [0], finish=False)
            if sm == 2:
                emit_proj(1, [0], finish=False)
        emit_proj(0, [1], finish=False)
        emit_proj(1, [1], finish=False)
        emit_proj(0, [2], finish=False)
        emit_proj(1, [2], finish=False)
        emit_proj(2, [0], finish=False, pool=psB, tag="att")
        emit_proj(2, [1], finish=False, pool=psB, tag="att")
        # the rest is emitted after the sm3 normalize chains above, so these
        # matmuls execute during the h2 DRAM round-trip and keep the PE warm
        emit_proj(3, [0], finish=False, pool=psB, tag="att")
        emit_proj(2, [2], finish=False, pool=psB, tag="att")
        emit_proj(3, [1], finish=False, pool=psB, tag="att")
        emit_proj(3, [2], finish=False, pool=psB, tag="att")

        # ---- proj finish ----
        emit_proj(0, [3], finish=True)
        emit_proj(1, [3], finish=True)
        emit_proj(2, [3], finish=True)
        emit_proj(3, [3], finish=True)

    nc.compile()
    return nc


def _host_prep(x, norm_gamma, norm_beta, qkv_w, qkv_b, proj_w, proj_b):
    x = np.asarray(x, dtype=np.float32).reshape(B, C, HW)
    qkv_w = np.asarray(qkv_w, dtype=np.float32)
    qkv_b = np.asarray(qkv_b, dtype=np.float32)
    proj_w = np.asarray(proj_w, dtype=np.float32)
    proj_b = np.asarray(proj_b, dtype=np.float32)

    wq_np = np.ascontiguousarray(qkv_w.T)
    if FP8_CONV:
        wqkvT = np.clip(wq_np, -440.0, 440.0).astype(ml_dtypes.float8_e4m3fn)
    else:
        wqkvT = wq_np.astype(ml_dtypes.bfloat16)
    wprojT = np.ascontiguousarray(proj_w.T).astype(ml_dtypes.bfloat16)
    qkb = np.ascontiguousarray(qkv_b[:2 * C])
    vb = np.ascontiguousarray(qkv_b[2 * C:]).astype(ml_dtypes.bfloat16)
    pb = np.ascontiguousarray(proj_b, dtype=np.float32)

    G = np.zeros((128, GPT), np.float32)
    for p in range(128):
        G[p, p // CPG] = 1.0 / CPG
    GT = np.zeros((8, 128), np.float32)
    for p in range(128):
        GT[p // CPG, p] = 1.0

    gamma = np.asarray(norm_gamma, dtype=np.float32)
    beta = np.asarray(norm_beta, dtype=np.float32)
    # packed per-partition constants [128, 28]:
    # cols 0-3 gamma, 4-7 beta, 8-15 qkb, 16-19 proj_b, 20-27 G
    cpack = np.zeros((128, 28), np.float32)
    cpack[:, 0:4] = gamma.reshape(CT, 128).T
    cpack[:, 4:8] = beta.reshape(CT, 128).T
    cpack[:, 8:16] = qkb.reshape(2 * CT, 128).T
    cpack[:, 16:20] = pb.reshape(CT, 128).T
    cpack[:, 20:28] = G
    cpack = np.ascontiguousarray(cpack)

    in_maps = []
    for b in range(B):
        in_maps.append({
            "x": np.ascontiguousarray(x[b]).astype(ml_dtypes.bfloat16),
            "wqkvT": wqkvT, "wprojT": wprojT,
            "cpack": cpack, "vb": vb, "GT": GT,
        })
    return in_maps


def _run(inputs: dict, trace: bool = False, tmpdir=None):
    if "nc" not in _CACHE:
        _CACHE["nc"] = _build()
    nc = _CACHE["nc"]
    in_maps = _host_prep(**inputs)
    res = run_bass_kernel_spmd(nc, in_maps, core_ids=list(range(8)), trace=trace,
                               tmpdir=tmpdir)
    out = np.stack([np.asarray(r["out"]).astype(np.float32)
                    for r in res.results]).reshape(B, C, 32, 32)
    return out, res


def kernel(**inputs):
    out, _ = _run(inputs, trace=False)
    return out
